# revision 7
# baseline (speedup 1.0000x reference)
"""Trainium2 Bass kernel for NeuralDecisionTree (soft decision tree MoE).

Strategy: data-parallel over batch across 8 NeuronCores (1024 rows/core),
weights replicated.  All heavy GEMMs run in bf16 (~halves weight-load
time, DMA bytes and PE power vs fp32r; ~5e-3 rel error, well within
tolerance).

TRN2 PE pays a ~90-110ns reconfiguration stall whenever consecutive
matmuls change tile shape or dtype (measured: 128x128->128x128 issues at
~220ns, any shape/dtype change ~310-340ns).  So every stationary operand
in the steady state is zero-padded to a [128,128] bf16 tile:
  - W2 per-leaf variants [W2_a|0] / [0|W2_b] accumulate both leaves of a
    pair into one [128,N] PSUM bank (start=True / start=False).
  - W3 pair pack padded 32->128 cols (pred rows 16..127 are zeros).
  - router weights padded 63->128 cols.
  - leaf probabilities: pT = exp(A64 @ S) once per batch tile (the only
    2 fp32r matmuls), stored bf16 with rows 64..127 zeroed; per-group
    broadcast into the pred slot layout via 0/1 selection matmuls
    BSEL_g @ pT (bf16, exact).
  - output mix: R^T and b3^T padded to [128,128] with the real columns
    at offset 32*t, all accumulating into one PSUM bank held open across
    the whole kernel (b3 terms start the group, 16 R-folds accumulate).

Per-core dataflow (activations kept in [feature, batch] layout):
  router:  z = router_W @ x^T            (4 K-chunk bf16 matmuls)
  S      = [ln s; 0; ln(1-s); 0]         (128 rows; s = sigmoid(z + rb))
  pT     = exp(A @ S)                    [64 leaves, N] -> bf16
  L1:     h1T_l = relu(W1_l^T @ x^T + b1)  4 K-chunk bf16 matmuls -> ACT
  L2:     pair into one [128,N] PSUM bank, single DVE bias+relu -> bf16
  L3:     pred pair via padded block-diagonal W3 pack -> [128,N] PSUM
  mix:    prod = pred * broadcast(pT) (DVE), out_ps += R_t^T @ prod.
"""

import os
import sys

import numpy as np

if "/opt/trn_rl_repo" not in sys.path:
    sys.path.insert(0, "/opt/trn_rl_repo")

import ml_dtypes

import concourse.bass as bass
import concourse.hw_specs as hw_specs
import concourse.tile as tile
from concourse import bacc, mybir
from concourse.bass_utils import run_bass_kernel_spmd

_ONE_TABLE = "natural_log_exp_and_others"
_orig_get_tables = hw_specs.get_activation_tables


def _patched_get_tables(module_arch):
    """Confine activation-table choice to one set that covers every ACT
    func this kernel uses (exp/ln/relu/abs/copy/identity), so the greedy
    per-instruction table picker never ping-pongs between sets.  Dict
    order (= act_func_set_id) is preserved; other sets are emptied."""
    tables = dict(_orig_get_tables(module_arch))
    keep = tables[_ONE_TABLE]
    return {k: (v if k == _ONE_TABLE else (v & set()) or set())
            if k != _ONE_TABLE else keep for k, v in tables.items()}

f32 = mybir.dt.float32
f32r = mybir.dt.float32r
bf16 = mybir.dt.bfloat16
AF = mybir.ActivationFunctionType
ALU = mybir.AluOpType

# Problem shape (hardcoded; harness contract)
B = 8192
D = 512
H1 = 128
H2 = 64
OUT = 8
L = 64
NI = 63
NCORES = 8
BC = B // NCORES        # 1024 rows per core
N = 512                 # batch tile (matmul free dim / PSUM bank)
T = BC // N             # 2 batch tiles per core
KC = D // 128           # 4 contraction chunks for the input dim
NPAIR = L // 2          # 32 leaf pairs
NG = 8                  # 8-leaf groups


def _leaf_path_rows(leaf):
    """Rows of the [128] log-sigmoid stack contributing to log p(leaf).

    Row n (n<63) holds ln d_n; row 64+n holds ln(1-d_n); rows 63 and 127
    are zero pads.  Mirrors the reference's level-wise p interleave.
    """
    rows = []
    for k in range(6):
        prefix = leaf >> (6 - k)
        node = (2 ** k - 1) + prefix
        bit = (leaf >> (5 - k)) & 1
        rows.append(node + 64 * bit)
    return rows


def build_nc():
    nc = bacc.Bacc("TRN2", target_bir_lowering=False, debug=False,
                   num_devices=NCORES)
    bacc_mod = sys.modules["concourse.bacc"]
    bacc_mod.get_activation_tables = _patched_get_tables

    d_xa = nc.dram_tensor("xa", [128, T, KC, N], bf16, kind="ExternalInput").ap()
    d_w1 = nc.dram_tensor("w1a", [128, L, KC, 128], bf16, kind="ExternalInput").ap()
    d_rw = nc.dram_tensor("rwa", [128, KC, 128], bf16, kind="ExternalInput").ap()
    d_w2 = nc.dram_tensor("w2a", [128, NPAIR, 2, 128], bf16, kind="ExternalInput").ap()
    d_w3 = nc.dram_tensor("w3p", [128, NPAIR, 128], bf16, kind="ExternalInput").ap()
    d_a64 = nc.dram_tensor("a64", [128, L], f32r, kind="ExternalInput").ap()
    d_bsel = nc.dram_tensor("bsel", [128, NG, 128], bf16, kind="ExternalInput").ap()
    d_r = nc.dram_tensor("rsel2", [128, T, 128], bf16, kind="ExternalInput").ap()
    d_b3 = nc.dram_tensor("b3e", [128, T, 128], bf16, kind="ExternalInput").ap()
    d_b1 = nc.dram_tensor("b1a", [128, L], f32, kind="ExternalInput").ap()
    d_b2 = nc.dram_tensor("b2a", [128, NPAIR], f32, kind="ExternalInput").ap()
    d_rbp = nc.dram_tensor("rbp", [64, 1], f32, kind="ExternalInput").ap()
    d_rbn = nc.dram_tensor("rbn", [64, 1], f32, kind="ExternalInput").ap()
    d_out = nc.dram_tensor("outT", [OUT, BC], f32, kind="ExternalOutput").ap()

    with tile.TileContext(nc) as tc:
        with tc.tile_pool(name="const", bufs=1) as cpool, \
             tc.tile_pool(name="w1", bufs=2) as w1pool, \
             tc.tile_pool(name="w2w3", bufs=3) as h2wpool, \
             tc.tile_pool(name="spool", bufs=2) as spool, \
             tc.tile_pool(name="parr", bufs=3) as papool, \
             tc.tile_pool(name="h1", bufs=4) as h1pool, \
             tc.tile_pool(name="h2", bufs=3) as h2pool, \
             tc.tile_pool(name="prod", bufs=3) as prpool, \
             tc.tile_pool(name="osb", bufs=2) as opool, \
             tc.tile_pool(name="ps_h1", bufs=3, space="PSUM") as ps_h1, \
             tc.tile_pool(name="ps_h2", bufs=2, space="PSUM") as ps_h2, \
             tc.tile_pool(name="ps_pm", bufs=2, space="PSUM") as ps_pm, \
             tc.tile_pool(name="ps_out", bufs=1, space="PSUM") as ps_out:

            # ---- constants into SBUF, split across the three DMA-issuing
            # queues so the critical first bytes land in parallel:
            #   sync (SP):   router weights -> x -> W1 stream (the bulk)
            #   gpsimd:      W2/W3 group tiles + BSEL
            #   scalar:      biases + small mix matrices
            rwa = cpool.tile([128, KC, 128], bf16)
            nc.sync.dma_start(rwa[:], d_rw)
            xa = cpool.tile([128, T, KC, N], bf16)
            for c in range(KC):
                nc.sync.dma_start(xa[:, 0, c, :], d_xa[:, 0, c, :])
            w1g0 = w1pool.tile([128, 8, KC, 128], bf16, tag="w1", name="w1g0")
            nc.sync.dma_start(w1g0[:, 0:2, :, :], d_w1[:, 0:2, :, :])
            for c in range(KC):
                nc.sync.dma_start(xa[:, 1, c, :], d_xa[:, 1, c, :])
            nc.sync.dma_start(w1g0[:, 2:4, :, :], d_w1[:, 2:4, :, :])
            nc.sync.dma_start(w1g0[:, 4:6, :, :], d_w1[:, 4:6, :, :])
            nc.sync.dma_start(w1g0[:, 6:8, :, :], d_w1[:, 6:8, :, :])

            w2g0 = h2wpool.tile([128, 4, 2, 128], bf16, tag="w2", name="w2g0")
            nc.gpsimd.dma_start(w2g0[:], d_w2[:, 0:4, :, :])
            bsel = cpool.tile([128, NG, 128], bf16)
            nc.gpsimd.dma_start(bsel[:], d_bsel)
            w3g0 = h2wpool.tile([128, 4, 128], bf16, tag="w3", name="w3g0")
            nc.gpsimd.dma_start(w3g0[:], d_w3[:, 0:4, :])

            rbp = cpool.tile([64, 1], f32)
            nc.scalar.dma_start(rbp[:], d_rbp)
            rbn = cpool.tile([64, 1], f32)
            nc.scalar.dma_start(rbn[:], d_rbn)
            b1a = cpool.tile([128, L], f32)
            nc.scalar.dma_start(b1a[:], d_b1)
            a64 = cpool.tile([128, L], f32r)
            nc.scalar.dma_start(a64[:], d_a64)
            b2a = cpool.tile([128, NPAIR], f32)
            nc.scalar.dma_start(b2a[:], d_b2)
            rsel2 = cpool.tile([128, T, 128], bf16)
            nc.scalar.dma_start(rsel2[:], d_r)
            b3e = cpool.tile([128, T, 128], bf16)
            nc.scalar.dma_start(b3e[:], d_b3)

            # ---- routing + first L1 pair, interleaved for startup ----
            # PE order: z(t0) -> L1 pair0 t0 (needs only the first W1 chunk)
            # -> z(t1) (by which time the second half of x has landed).
            # Row 63 of z is a zero pad from the padded router weights.
            #   ln s     = -(relu(-z') + ln(1 + exp(-|z'|)))
            #   ln (1-s) = -(relu( z') + ln(1 + exp(-|z'|)))
            def emit_l1_leaf(s, t, jj, e, w1g):
                leaf = 2 * s + e
                h1_ps = ps_h1.tile([128, N], f32, tag="h1",
                                   name=f"h1ps{s}_{t}_{e}")
                for c in range(KC):
                    nc.tensor.matmul(
                        h1_ps[:], w1g[:, 2 * jj + e, c, :],
                        xa[:, t, c, :],
                        start=(c == 0), stop=(c == KC - 1))
                h1_t = h1pool.tile([128, N], bf16, tag="h1s",
                                   name=f"h1s{s}_{t}_{e}")
                nc.scalar.activation(h1_t[:], h1_ps[:], AF.Relu,
                                     bias=b1a[:, leaf:leaf + 1],
                                     scale=1.0)
                return h1_t

            def emit_l1(s, t, jj, w1g):
                return [emit_l1_leaf(s, t, jj, e, w1g) for e in range(2)]

            z_pss = []
            pre_h1 = {}
            for t in range(T):
                z_ps = ps_pm.tile([128, N], f32, tag="pm", name=f"z_ps{t}")
                for c in range(KC):
                    nc.tensor.matmul(z_ps[:], rwa[:, c, :],
                                     xa[:, t, c, :],
                                     start=(c == 0), stop=(c == KC - 1))
                z_pss.append(z_ps)
                if t == 0:
                    pre_h1[(0, 0)] = emit_l1(0, 0, 0, w1g0)
            qs, rzps, rzns = [], [], []
            for t in range(T):
                z_ps = z_pss[t]
                az = spool.tile([64, N], f32, tag="az", name=f"az{t}")
                nc.scalar.activation(az[:], z_ps[0:64, :], AF.Abs,
                                     bias=rbp[:], scale=1.0)
                e_t = spool.tile([64, N], f32, tag="e", name=f"e{t}")
                nc.scalar.activation(e_t[:], az[:], AF.Exp, scale=-1.0)
                rzp = spool.tile([64, N], f32, tag="rzp", name=f"rzp{t}")
                nc.scalar.activation(rzp[:], z_ps[0:64, :], AF.Relu,
                                     bias=rbp[:], scale=1.0)
                rzn = spool.tile([64, N], f32, tag="rzn", name=f"rzn{t}")
                nc.scalar.activation(rzn[:], z_ps[0:64, :], AF.Relu,
                                     bias=rbn[:], scale=-1.0)
                qs.append(e_t)
                rzps.append(rzp)
                rzns.append(rzn)
            s_tiles = []
            for t in range(T):
                q_t = qs[t]
                nc.scalar.activation(q_t[:], q_t[:], AF.Ln, bias=1.0,
                                     scale=1.0)
                s_t = spool.tile([128, N], f32r, tag="s", name=f"s{t}")
                nc.vector.scalar_tensor_tensor(
                    s_t[0:64, :], rzns[t][:], -1.0, q_t[:],
                    op0=ALU.mult, op1=ALU.subtract)
                nc.vector.scalar_tensor_tensor(
                    s_t[64:128, :], rzps[t][:], -1.0, q_t[:],
                    op0=ALU.mult, op1=ALU.subtract)
                s_tiles.append(s_t)

            # ---- leaf probabilities: pT = exp(A @ S) once per batch tile
            # (the only fp32r matmuls in the kernel), held in bf16 with
            # rows 64..127 zeroed so the bf16 BSEL broadcasts can contract
            # the full 128 partitions.
            pt_tiles = []
            for t in range(T):
                pt_ext = spool.tile([128, N], bf16, tag="ptx", name=f"ptx{t}")
                nc.vector.memset(pt_ext[64:128, :], 0.0)
                pt_ps = ps_pm.tile([64, N], f32, tag="pm", name=f"pt_ps{t}")
                nc.tensor.matmul(pt_ps[:], a64[:], s_tiles[t][:],
                                 start=True, stop=True)
                nc.scalar.activation(pt_ext[0:64, :], pt_ps[:], AF.Exp,
                                     scale=1.0)
                pt_tiles.append(pt_ext)

            # ---- per-batch-tile output accumulator: one PSUM bank, t0's
            # real rows at partition base 0 and t1's at base 32 (the padded
            # R/b3 stationaries place their real columns at 32*t).  The
            # accumulation group stays open across all 8 groups; writes are
            # full-bank with zeros outside each tile's real rows, so
            # interleaving is harmless.  b3 terms run first: t0 start=True
            # zeroes the bank, t1 accumulates.
            out_ps = ps_out.tile([128, N], f32, tag="out", name="out_ps")

            # ---- main loop over 8-leaf groups ----
            def emit_p_block(g, t):
                """BSEL broadcast of pT into this group's pred slot layout
                (bf16 0/1 matmul, exact), copied to SBUF."""
                pa_ps = ps_pm.tile([128, N], f32, tag="pm",
                                   name=f"pa_ps{g}_{t}")
                nc.tensor.matmul(pa_ps[:], bsel[:, g, :], pt_tiles[t][:],
                                 start=True, stop=True)
                pa_t = papool.tile([128, N], bf16, tag="pa", name=f"pa{g}_{t}")
                nc.scalar.activation(pa_t[:], pa_ps[:], AF.Copy, scale=1.0)
                return pa_t

            first_rsel = [True, True]
            for g in range(NG):
                if g == 0:
                    w1g, w2g, w3g = w1g0, w2g0, w3g0
                else:
                    w2g = h2wpool.tile([128, 4, 2, 128], bf16, tag="w2",
                                       name=f"w2g{g}")
                    nc.gpsimd.dma_start(w2g[:], d_w2[:, 4 * g:4 * (g + 1), :, :])
                    w3g = h2wpool.tile([128, 4, 128], bf16, tag="w3",
                                       name=f"w3g{g}")
                    nc.gpsimd.dma_start(w3g[:], d_w3[:, 4 * g:4 * (g + 1), :])
                    w1g = w1pool.tile([128, 8, KC, 128], bf16, tag="w1",
                                      name=f"w1g{g}")
                    for hc in range(2):
                        nc.sync.dma_start(
                            w1g[:, 4 * hc:4 * (hc + 1), :, :],
                            d_w1[:, 8 * g + 4 * hc:8 * g + 4 * (hc + 1), :, :])
                # For g>0, p broadcasts can be computed up front (pT is
                # long done).  For g=0 they are emitted mid-way through the
                # first pair so the PE queue never waits on the routing
                # ACT/DVE chain.
                if g > 0:
                    pa_ts = [emit_p_block(g, t) for t in range(T)]
                else:
                    pa_ts = [None, None]
                prod_ts = [prpool.tile([128, N], bf16, tag="prod",
                                       name=f"prod{g}_{t}") for t in range(T)]
                def emit_l2(s, t, jj, h1_pair):
                    """Both leaves of the pair into one [128,N] PSUM bank
                    via padded [W2_a|0] / [0|W2_b] stationaries (PSUM
                    accumulation), one fused bias+relu DVE."""
                    h2_ps = ps_h2.tile([128, N], f32, tag="h2",
                                       name=f"h2ps{s}_{t}")
                    for e in range(2):
                        nc.tensor.matmul(h2_ps[:], w2g[:, jj, e, :],
                                         h1_pair[e][:],
                                         start=(e == 0), stop=(e == 1))
                    h2pair = h2pool.tile([128, N], bf16, tag="h2s",
                                         name=f"h2p{s}_{t}")
                    nc.vector.tensor_scalar(
                        h2pair[:], h2_ps[:], b2a[:, s:s + 1], 0.0,
                        op0=ALU.add, op1=ALU.max)
                    return h2pair

                def emit_l3(s, t, jj, h2pair):
                    pred_ps = ps_pm.tile([128, N], f32, tag="pm",
                                         name=f"pred{s}_{t}")
                    nc.tensor.matmul(pred_ps[:], w3g[:, jj, :], h2pair[:],
                                     start=True, stop=True)
                    if g == 0 and jj == 0:
                        pa_ts[t] = emit_p_block(0, t)
                        if t == 1:
                            # b3 terms open the out accumulation group as
                            # soon as both pT tiles exist (before any R
                            # fold): t0 zeroes the bank, t1 accumulates.
                            for tt in range(T):
                                nc.tensor.matmul(
                                    out_ps[:], b3e[:, tt, :],
                                    pt_tiles[tt][:],
                                    start=(tt == 0), stop=False,
                                    skip_group_check=True)
                    nc.vector.tensor_mul(
                        prod_ts[t][32 * jj:32 * (jj + 1), :],
                        pred_ps[0:32, :],
                        pa_ts[t][32 * jj:32 * (jj + 1), :])

                for jj in range(4):
                    s = 4 * g + jj
                    # pipelined order: L1(t0) -> L1(t1,A) -> L2(t0) ->
                    # L1(t1,B) -> L3(t0) -> L2(t1) -> L3(t1); keeps ACT's
                    # relus and DVE's h2 ops ahead of their PE consumers
                    # while holding at most 3 h1 PSUM banks.
                    if (s, 0) in pre_h1:
                        h1_t0 = pre_h1[(s, 0)]
                    else:
                        h1_t0 = emit_l1(s, 0, jj, w1g)
                    h1_t1 = [emit_l1_leaf(s, 1, jj, 0, w1g)]
                    h2_t0 = emit_l2(s, 0, jj, h1_t0)
                    h1_t1.append(emit_l1_leaf(s, 1, jj, 1, w1g))
                    emit_l3(s, 0, jj, h2_t0)
                    h2_t1 = emit_l2(s, 1, jj, h1_t1)
                    emit_l3(s, 1, jj, h2_t1)
                for t in range(T):
                    nc.tensor.matmul(out_ps[:], rsel2[:, t, :],
                                     prod_ts[t][:],
                                     start=False,
                                     stop=(g == NG - 1 and t == T - 1),
                                     skip_group_check=True)

            # ---- write out (copies on the idle scalar engine) ----
            for t in range(T):
                out_sb = opool.tile([OUT, N], f32, tag="o", name=f"osb{t}")
                nc.scalar.activation(out_sb[:], out_ps[32 * t:32 * t + OUT, :],
                                     AF.Copy)
                nc.sync.dma_start(d_out[:, N * t:N * (t + 1)], out_sb[:])

    try:
        nc.compile()
    finally:
        bacc_mod.get_activation_tables = _orig_get_tables
    return nc


def pack_shared(router_W, router_b, W1, b1, W2, b2, W3, b3):
    """Host-side packing of replicated parameters into SBUF-friendly layouts."""
    f = np.float32
    b16 = ml_dtypes.bfloat16
    router_W = np.asarray(router_W, f)
    router_b = np.asarray(router_b, f)
    W1 = np.asarray(W1, f)
    b1 = np.asarray(b1, f)
    W2 = np.asarray(W2, f)
    b2 = np.asarray(b2, f)
    W3 = np.asarray(W3, f)
    b3 = np.asarray(b3, f)

    w1a = np.ascontiguousarray(W1.reshape(L, KC, 128, H1).transpose(2, 0, 1, 3))
    rwa = np.zeros((128, KC, 128), f)
    rwa[:, :, 0:NI] = router_W.T.reshape(KC, 128, NI).transpose(1, 0, 2)

    # W2 padded variants: e=0 -> [W2_a | 0], e=1 -> [0 | W2_b], so both
    # leaves of a pair accumulate into one [128,N] PSUM bank.
    w2a = np.zeros((NPAIR, 2, 128, 128), f)
    for s in range(NPAIR):
        w2a[s, 0, :, 0:64] = W2[2 * s]
        w2a[s, 1, :, 64:128] = W2[2 * s + 1]
    w2a = np.ascontiguousarray(w2a.transpose(2, 0, 1, 3))

    w3p = np.zeros((NPAIR, 128, 128), f)
    for s in range(NPAIR):
        w3p[s, 0:64, 0:8] = W3[2 * s]
        w3p[s, 64:128, 8:16] = W3[2 * s + 1]
    w3p = np.ascontiguousarray(w3p.transpose(1, 0, 2))

    a64 = np.zeros((128, L), f)
    for leaf in range(L):
        for row in _leaf_path_rows(leaf):
            a64[row, leaf] += 1.0

    # BSEL: broadcast pT [64 leaves] into the per-group pred slot layout
    # (32-row blocks per pair: 8 slots leaf_a, 8 slots leaf_b, 16 zeros).
    bsel = np.zeros((128, NG, 128), f)
    for g in range(NG):
        for jj in range(4):
            for m in range(16):
                leaf = 2 * (4 * g + jj) + (m >= 8)
                bsel[leaf, g, 32 * jj + m] = 1.0

    # R fold / b3 terms, padded to [128,128] with the real columns at
    # offset 32*t so both batch tiles share one PSUM accumulator bank.
    rsel2 = np.zeros((128, T, 128), f)
    for t in range(T):
        for kk in range(128):
            m = kk % 32
            if m < 8:
                rsel2[kk, t, 32 * t + m] = 1.0
            elif m < 16:
                rsel2[kk, t, 32 * t + (m - 8)] = 1.0
    b3e = np.zeros((128, T, 128), f)
    for t in range(T):
        b3e[0:L, t, 32 * t:32 * t + OUT] = b3

    return {
        "w1a": w1a.astype(b16),
        "rwa": rwa.astype(b16),
        "w2a": w2a.astype(b16),
        "w3p": w3p.astype(b16),
        "a64": a64,
        "bsel": bsel.astype(b16),
        "rsel2": rsel2.astype(b16),
        "b3e": b3e.astype(b16),
        "b1a": np.ascontiguousarray(b1.T),
        "b2a": np.ascontiguousarray(b2.reshape(NPAIR, 128).T),
        "rbp": np.concatenate([router_b, [0.0]]).astype(f)[:, None],
        "rbn": np.concatenate([-router_b, [0.0]]).astype(f)[:, None],
    }


def pack_x_core(x_core):
    """[1024, 512] slice -> [128, T, KC, 512] tile-major transposed bf16."""
    xc = np.asarray(x_core, np.float32)
    parts = []
    for t in range(T):
        parts.append(xc[N * t:N * (t + 1)].T.reshape(KC, 128, N))
    stacked = np.stack(parts, axis=0)            # [T, KC, 128, N]
    return np.ascontiguousarray(
        stacked.transpose(2, 0, 1, 3)).astype(ml_dtypes.bfloat16)


_NC_CACHE = {}


def _get_nc():
    if "nc" not in _NC_CACHE:
        _NC_CACHE["nc"] = build_nc()
    return _NC_CACHE["nc"]


def kernel(**inputs):
    x = np.asarray(inputs["x"], np.float32)
    shared = pack_shared(inputs["router_W"], inputs["router_b"],
                         inputs["W1"], inputs["b1"], inputs["W2"],
                         inputs["b2"], inputs["W3"], inputs["b3"])
    in_maps = []
    for i in range(NCORES):
        m = dict(shared)
        m["xa"] = pack_x_core(x[BC * i:BC * (i + 1)])
        in_maps.append(m)
    nc = _get_nc()
    res = run_bass_kernel_spmd(nc, in_maps, core_ids=list(range(NCORES)))
    out = np.concatenate([r["outT"].T for r in res.results], axis=0)
    return np.ascontiguousarray(out, np.float32)


# revision 9
# speedup vs baseline: 1.0235x; 1.0235x over previous
"""Trainium2 Bass kernel for NeuralDecisionTree (soft decision tree MoE).

Strategy: data-parallel over batch across 8 NeuronCores (1024 rows/core),
weights replicated.  All heavy GEMMs run in bf16 (~halves weight-load
time, DMA bytes and PE power vs fp32r; ~5e-3 rel error, well within
tolerance).

TRN2 PE pays a ~90-110ns reconfiguration stall whenever consecutive
matmuls change tile shape or dtype (measured: 128x128->128x128 issues at
~220ns, any shape/dtype change ~310-340ns).  So every stationary operand
in the steady state is zero-padded to a [128,128] bf16 tile:
  - W2 per-leaf variants [W2_a|0] / [0|W2_b] accumulate both leaves of a
    pair into one [128,N] PSUM bank (start=True / start=False).
  - W3 pair pack padded 32->128 cols (pred rows 16..127 are zeros).
  - router weights padded 63->128 cols.
  - leaf probabilities: pT = exp(A64 @ S) once per batch tile (the only
    2 fp32r matmuls), stored bf16 with rows 64..127 zeroed; per-group
    broadcast into the pred slot layout via 0/1 selection matmuls
    BSEL_g @ pT (bf16, exact).
  - output mix: R^T and b3^T padded to [128,128] with the real columns
    at offset 32*t, all accumulating into one PSUM bank held open across
    the whole kernel (b3 terms start the group, 16 R-folds accumulate).

Per-core dataflow (activations kept in [feature, batch] layout):
  router:  z = router_W @ x^T            (4 K-chunk bf16 matmuls)
  S      = [ln s; 0; ln(1-s); 0]         (128 rows; s = sigmoid(z + rb))
  pT     = exp(A @ S)                    [64 leaves, N] -> bf16
  L1:     h1T_l = relu(W1_l^T @ x^T + b1)  4 K-chunk bf16 matmuls -> ACT
  L2:     pair into one [128,N] PSUM bank, single DVE bias+relu -> bf16
  L3:     pred pair via padded block-diagonal W3 pack -> [128,N] PSUM
  mix:    prod = pred * broadcast(pT) (DVE), out_ps += R_t^T @ prod.
"""

import os
import sys

import numpy as np

if "/opt/trn_rl_repo" not in sys.path:
    sys.path.insert(0, "/opt/trn_rl_repo")

import ml_dtypes

import concourse.bass as bass
import concourse.hw_specs as hw_specs
import concourse.tile as tile
from concourse import bacc, mybir
from concourse.bass_utils import run_bass_kernel_spmd

_ONE_TABLE = "natural_log_exp_and_others"
_orig_get_tables = hw_specs.get_activation_tables


def _patched_get_tables(module_arch):
    """Confine activation-table choice to one set that covers every ACT
    func this kernel uses (exp/ln/relu/abs/copy/identity), so the greedy
    per-instruction table picker never ping-pongs between sets.  Dict
    order (= act_func_set_id) is preserved; other sets are emptied."""
    tables = dict(_orig_get_tables(module_arch))
    keep = tables[_ONE_TABLE]
    return {k: (v if k == _ONE_TABLE else (v & set()) or set())
            if k != _ONE_TABLE else keep for k, v in tables.items()}

f32 = mybir.dt.float32
f32r = mybir.dt.float32r
bf16 = mybir.dt.bfloat16
AF = mybir.ActivationFunctionType
ALU = mybir.AluOpType

# Problem shape (hardcoded; harness contract)
B = 8192
D = 512
H1 = 128
H2 = 64
OUT = 8
L = 64
NI = 63
NCORES = 8
BC = B // NCORES        # 1024 rows per core
N = 512                 # batch tile (matmul free dim / PSUM bank)
T = BC // N             # 2 batch tiles per core
KC = D // 128           # 4 contraction chunks for the input dim
NPAIR = L // 2          # 32 leaf pairs
NG = 8                  # 8-leaf groups


def _leaf_path_rows(leaf):
    """Rows of the [128] log-sigmoid stack contributing to log p(leaf).

    Row n (n<63) holds ln d_n; row 64+n holds ln(1-d_n); rows 63 and 127
    are zero pads.  Mirrors the reference's level-wise p interleave.
    """
    rows = []
    for k in range(6):
        prefix = leaf >> (6 - k)
        node = (2 ** k - 1) + prefix
        bit = (leaf >> (5 - k)) & 1
        rows.append(node + 64 * bit)
    return rows


def build_nc():
    nc = bacc.Bacc("TRN2", target_bir_lowering=False, debug=False,
                   num_devices=NCORES)
    bacc_mod = sys.modules["concourse.bacc"]
    bacc_mod.get_activation_tables = _patched_get_tables

    d_xa = nc.dram_tensor("xa", [128, T, KC, N], bf16, kind="ExternalInput").ap()
    d_w1 = nc.dram_tensor("w1a", [128, L, KC, 128], bf16, kind="ExternalInput").ap()
    d_rw = nc.dram_tensor("rwa", [128, KC, 128], bf16, kind="ExternalInput").ap()
    d_w2 = nc.dram_tensor("w2a", [128, NPAIR, 2, 128], bf16, kind="ExternalInput").ap()
    d_w3 = nc.dram_tensor("w3p", [128, NPAIR, 128], bf16, kind="ExternalInput").ap()
    d_a64 = nc.dram_tensor("a64", [128, L], f32r, kind="ExternalInput").ap()
    d_bsel = nc.dram_tensor("bsel", [128, NG, 128], bf16, kind="ExternalInput").ap()
    d_r = nc.dram_tensor("rsel2", [128, T, 128], bf16, kind="ExternalInput").ap()
    d_b3 = nc.dram_tensor("b3e", [128, T, 128], bf16, kind="ExternalInput").ap()
    d_b1 = nc.dram_tensor("b1a", [128, L], f32, kind="ExternalInput").ap()
    d_b2 = nc.dram_tensor("b2a", [128, NPAIR], f32, kind="ExternalInput").ap()
    d_rbp = nc.dram_tensor("rbp", [64, 1], f32, kind="ExternalInput").ap()
    d_rbn = nc.dram_tensor("rbn", [64, 1], f32, kind="ExternalInput").ap()
    d_out = nc.dram_tensor("outT", [OUT, BC], f32, kind="ExternalOutput").ap()

    with tile.TileContext(nc) as tc:
        with tc.tile_pool(name="const", bufs=1) as cpool, \
             tc.tile_pool(name="w1", bufs=2) as w1pool, \
             tc.tile_pool(name="w2w3", bufs=3) as h2wpool, \
             tc.tile_pool(name="spool", bufs=2) as spool, \
             tc.tile_pool(name="parr", bufs=3) as papool, \
             tc.tile_pool(name="h1", bufs=4) as h1pool, \
             tc.tile_pool(name="h2", bufs=3) as h2pool, \
             tc.tile_pool(name="prod", bufs=3) as prpool, \
             tc.tile_pool(name="osb", bufs=2) as opool, \
             tc.tile_pool(name="ps_h1", bufs=3, space="PSUM") as ps_h1, \
             tc.tile_pool(name="ps_h2", bufs=2, space="PSUM") as ps_h2, \
             tc.tile_pool(name="ps_pm", bufs=2, space="PSUM") as ps_pm, \
             tc.tile_pool(name="ps_out", bufs=1, space="PSUM") as ps_out:

            # ---- constants into SBUF, split across the three DMA-issuing
            # queues so the critical first bytes land in parallel:
            #   sync (SP):   router weights -> x -> W1 stream (the bulk)
            #   gpsimd:      W2/W3 group tiles + BSEL
            #   scalar:      biases + small mix matrices
            rwa = cpool.tile([128, KC, 128], bf16)
            nc.sync.dma_start(rwa[:], d_rw)
            xa = cpool.tile([128, T, KC, N], bf16)
            for c in range(KC):
                nc.sync.dma_start(xa[:, 0, c, :], d_xa[:, 0, c, :])
            w1g0 = w1pool.tile([128, 8, KC, 128], bf16, tag="w1", name="w1g0")
            nc.sync.dma_start(w1g0[:, 0:2, :, :], d_w1[:, 0:2, :, :])
            for c in range(KC):
                nc.sync.dma_start(xa[:, 1, c, :], d_xa[:, 1, c, :])
            nc.sync.dma_start(w1g0[:, 2:4, :, :], d_w1[:, 2:4, :, :])
            nc.sync.dma_start(w1g0[:, 4:6, :, :], d_w1[:, 4:6, :, :])
            nc.sync.dma_start(w1g0[:, 6:8, :, :], d_w1[:, 6:8, :, :])

            w2g0 = h2wpool.tile([128, 4, 2, 128], bf16, tag="w2", name="w2g0")
            nc.scalar.dma_start(w2g0[:], d_w2[:, 0:4, :, :])
            bsel = cpool.tile([128, NG, 128], bf16)
            nc.scalar.dma_start(bsel[:], d_bsel)
            w3g0 = h2wpool.tile([128, 4, 128], bf16, tag="w3", name="w3g0")
            nc.scalar.dma_start(w3g0[:], d_w3[:, 0:4, :])

            rbp = cpool.tile([64, 1], f32)
            nc.scalar.dma_start(rbp[:], d_rbp)
            rbn = cpool.tile([64, 1], f32)
            nc.scalar.dma_start(rbn[:], d_rbn)
            b1a = cpool.tile([128, L], f32)
            nc.scalar.dma_start(b1a[:], d_b1)
            a64 = cpool.tile([128, L], f32r)
            nc.scalar.dma_start(a64[:], d_a64)
            b2a = cpool.tile([128, NPAIR], f32)
            nc.scalar.dma_start(b2a[:], d_b2)
            rsel2 = cpool.tile([128, T, 128], bf16)
            nc.scalar.dma_start(rsel2[:], d_r)
            b3e = cpool.tile([128, T, 128], bf16)
            nc.scalar.dma_start(b3e[:], d_b3)

            # ---- routing + first L1 pair, interleaved for startup ----
            # PE order: z(t0) -> L1 pair0 t0 (needs only the first W1 chunk)
            # -> z(t1) (by which time the second half of x has landed).
            # Row 63 of z is a zero pad from the padded router weights.
            #   ln s     = -(relu(-z') + ln(1 + exp(-|z'|)))
            #   ln (1-s) = -(relu( z') + ln(1 + exp(-|z'|)))
            def emit_l1_leaf(s, t, jj, e, w1g):
                leaf = 2 * s + e
                h1_ps = ps_h1.tile([128, N], f32, tag="h1",
                                   name=f"h1ps{s}_{t}_{e}")
                for c in range(KC):
                    nc.tensor.matmul(
                        h1_ps[:], w1g[:, 2 * jj + e, c, :],
                        xa[:, t, c, :],
                        start=(c == 0), stop=(c == KC - 1))
                h1_t = h1pool.tile([128, N], bf16, tag="h1s",
                                   name=f"h1s{s}_{t}_{e}")
                nc.scalar.activation(h1_t[:], h1_ps[:], AF.Relu,
                                     bias=b1a[:, leaf:leaf + 1],
                                     scale=1.0)
                return h1_t

            def emit_l1(s, t, jj, w1g):
                return [emit_l1_leaf(s, t, jj, e, w1g) for e in range(2)]

            z_pss = []
            pre_h1 = {}
            for t in range(T):
                z_ps = ps_pm.tile([128, N], f32, tag="pm", name=f"z_ps{t}")
                for c in range(KC):
                    nc.tensor.matmul(z_ps[:], rwa[:, c, :],
                                     xa[:, t, c, :],
                                     start=(c == 0), stop=(c == KC - 1))
                z_pss.append(z_ps)
                if t == 0:
                    pre_h1[(0, 0)] = emit_l1(0, 0, 0, w1g0)
            qs, rzps, rzns = [], [], []
            for t in range(T):
                z_ps = z_pss[t]
                az = spool.tile([64, N], f32, tag="az", name=f"az{t}")
                nc.scalar.activation(az[:], z_ps[0:64, :], AF.Abs,
                                     bias=rbp[:], scale=1.0)
                e_t = spool.tile([64, N], f32, tag="e", name=f"e{t}")
                nc.scalar.activation(e_t[:], az[:], AF.Exp, scale=-1.0)
                rzp = spool.tile([64, N], f32, tag="rzp", name=f"rzp{t}")
                nc.scalar.activation(rzp[:], z_ps[0:64, :], AF.Relu,
                                     bias=rbp[:], scale=1.0)
                rzn = spool.tile([64, N], f32, tag="rzn", name=f"rzn{t}")
                nc.scalar.activation(rzn[:], z_ps[0:64, :], AF.Relu,
                                     bias=rbn[:], scale=-1.0)
                qs.append(e_t)
                rzps.append(rzp)
                rzns.append(rzn)
            s_tiles = []
            for t in range(T):
                q_t = qs[t]
                nc.scalar.activation(q_t[:], q_t[:], AF.Ln, bias=1.0,
                                     scale=1.0)
                s_t = spool.tile([128, N], f32r, tag="s", name=f"s{t}")
                nc.vector.scalar_tensor_tensor(
                    s_t[0:64, :], rzns[t][:], -1.0, q_t[:],
                    op0=ALU.mult, op1=ALU.subtract)
                nc.vector.scalar_tensor_tensor(
                    s_t[64:128, :], rzps[t][:], -1.0, q_t[:],
                    op0=ALU.mult, op1=ALU.subtract)
                s_tiles.append(s_t)

            # ---- leaf probabilities: pT = exp(A @ S) once per batch tile
            # (the only fp32r matmuls in the kernel), held in bf16 with
            # rows 64..127 zeroed so the bf16 BSEL broadcasts can contract
            # the full 128 partitions.
            pt_tiles = []
            for t in range(T):
                pt_ext = spool.tile([128, N], bf16, tag="ptx", name=f"ptx{t}")
                nc.vector.memset(pt_ext[64:128, :], 0.0)
                pt_ps = ps_pm.tile([64, N], f32, tag="pm", name=f"pt_ps{t}")
                nc.tensor.matmul(pt_ps[:], a64[:], s_tiles[t][:],
                                 start=True, stop=True)
                nc.scalar.activation(pt_ext[0:64, :], pt_ps[:], AF.Exp,
                                     scale=1.0)
                pt_tiles.append(pt_ext)

            # ---- per-batch-tile output accumulator: one PSUM bank, t0's
            # real rows at partition base 0 and t1's at base 32 (the padded
            # R/b3 stationaries place their real columns at 32*t).  The
            # accumulation group stays open across all 8 groups; writes are
            # full-bank with zeros outside each tile's real rows, so
            # interleaving is harmless.  b3 terms run first: t0 start=True
            # zeroes the bank, t1 accumulates.
            out_ps = ps_out.tile([128, N], f32, tag="out", name="out_ps")

            # ---- main loop over 8-leaf groups ----
            def emit_p_block(g, t):
                """BSEL broadcast of pT into this group's pred slot layout
                (bf16 0/1 matmul, exact), copied to SBUF."""
                pa_ps = ps_pm.tile([128, N], f32, tag="pm",
                                   name=f"pa_ps{g}_{t}")
                nc.tensor.matmul(pa_ps[:], bsel[:, g, :], pt_tiles[t][:],
                                 start=True, stop=True)
                pa_t = papool.tile([128, N], bf16, tag="pa", name=f"pa{g}_{t}")
                nc.scalar.activation(pa_t[:], pa_ps[:], AF.Copy, scale=1.0)
                return pa_t

            first_rsel = [True, True]
            for g in range(NG):
                if g == 0:
                    w1g, w2g, w3g = w1g0, w2g0, w3g0
                else:
                    w2g = h2wpool.tile([128, 4, 2, 128], bf16, tag="w2",
                                       name=f"w2g{g}")
                    nc.sync.dma_start(w2g[:], d_w2[:, 4 * g:4 * (g + 1), :, :])
                    w3g = h2wpool.tile([128, 4, 128], bf16, tag="w3",
                                       name=f"w3g{g}")
                    nc.sync.dma_start(w3g[:], d_w3[:, 4 * g:4 * (g + 1), :])
                    w1g = w1pool.tile([128, 8, KC, 128], bf16, tag="w1",
                                      name=f"w1g{g}")
                    for hc in range(2):
                        nc.sync.dma_start(
                            w1g[:, 4 * hc:4 * (hc + 1), :, :],
                            d_w1[:, 8 * g + 4 * hc:8 * g + 4 * (hc + 1), :, :])
                # For g>0, p broadcasts can be computed up front (pT is
                # long done).  For g=0 they are emitted mid-way through the
                # first pair so the PE queue never waits on the routing
                # ACT/DVE chain.
                if g > 0:
                    pa_ts = [emit_p_block(g, t) for t in range(T)]
                else:
                    pa_ts = [None, None]
                prod_ts = [prpool.tile([128, N], bf16, tag="prod",
                                       name=f"prod{g}_{t}") for t in range(T)]
                def emit_l2(s, t, jj, h1_pair):
                    """Both leaves of the pair into one [128,N] PSUM bank
                    via padded [W2_a|0] / [0|W2_b] stationaries (PSUM
                    accumulation), one fused bias+relu DVE."""
                    h2_ps = ps_h2.tile([128, N], f32, tag="h2",
                                       name=f"h2ps{s}_{t}")
                    for e in range(2):
                        nc.tensor.matmul(h2_ps[:], w2g[:, jj, e, :],
                                         h1_pair[e][:],
                                         start=(e == 0), stop=(e == 1))
                    h2pair = h2pool.tile([128, N], bf16, tag="h2s",
                                         name=f"h2p{s}_{t}")
                    nc.vector.tensor_scalar(
                        h2pair[:], h2_ps[:], b2a[:, s:s + 1], 0.0,
                        op0=ALU.add, op1=ALU.max)
                    return h2pair

                def emit_l3(s, t, jj, h2pair):
                    pred_ps = ps_pm.tile([128, N], f32, tag="pm",
                                         name=f"pred{s}_{t}")
                    nc.tensor.matmul(pred_ps[:], w3g[:, jj, :], h2pair[:],
                                     start=True, stop=True)
                    if g == 0 and jj == 0:
                        pa_ts[t] = emit_p_block(0, t)
                        if t == 1:
                            # b3 terms open the out accumulation group as
                            # soon as both pT tiles exist (before any R
                            # fold): t0 zeroes the bank, t1 accumulates.
                            for tt in range(T):
                                nc.tensor.matmul(
                                    out_ps[:], b3e[:, tt, :],
                                    pt_tiles[tt][:],
                                    start=(tt == 0), stop=False,
                                    skip_group_check=True)
                    nc.vector.tensor_mul(
                        prod_ts[t][32 * jj:32 * (jj + 1), :],
                        pred_ps[0:32, :],
                        pa_ts[t][32 * jj:32 * (jj + 1), :])

                for jj in range(4):
                    s = 4 * g + jj
                    # pipelined order: L1(t0) -> L1(t1,A) -> L2(t0) ->
                    # L1(t1,B) -> L3(t0) -> L2(t1) -> L3(t1); keeps ACT's
                    # relus and DVE's h2 ops ahead of their PE consumers
                    # while holding at most 3 h1 PSUM banks.
                    if (s, 0) in pre_h1:
                        h1_t0 = pre_h1[(s, 0)]
                    else:
                        h1_t0 = emit_l1(s, 0, jj, w1g)
                    h1_t1 = [emit_l1_leaf(s, 1, jj, 0, w1g)]
                    h2_t0 = emit_l2(s, 0, jj, h1_t0)
                    h1_t1.append(emit_l1_leaf(s, 1, jj, 1, w1g))
                    emit_l3(s, 0, jj, h2_t0)
                    h2_t1 = emit_l2(s, 1, jj, h1_t1)
                    emit_l3(s, 1, jj, h2_t1)
                for t in range(T):
                    nc.tensor.matmul(out_ps[:], rsel2[:, t, :],
                                     prod_ts[t][:],
                                     start=False,
                                     stop=(g == NG - 1 and t == T - 1),
                                     skip_group_check=True)

            # ---- write out (copies on the idle scalar engine) ----
            for t in range(T):
                out_sb = opool.tile([OUT, N], f32, tag="o", name=f"osb{t}")
                nc.scalar.activation(out_sb[:], out_ps[32 * t:32 * t + OUT, :],
                                     AF.Copy)
                nc.sync.dma_start(d_out[:, N * t:N * (t + 1)], out_sb[:])

    try:
        nc.compile()
    finally:
        bacc_mod.get_activation_tables = _orig_get_tables
    return nc


def pack_shared(router_W, router_b, W1, b1, W2, b2, W3, b3):
    """Host-side packing of replicated parameters into SBUF-friendly layouts."""
    f = np.float32
    b16 = ml_dtypes.bfloat16
    router_W = np.asarray(router_W, f)
    router_b = np.asarray(router_b, f)
    W1 = np.asarray(W1, f)
    b1 = np.asarray(b1, f)
    W2 = np.asarray(W2, f)
    b2 = np.asarray(b2, f)
    W3 = np.asarray(W3, f)
    b3 = np.asarray(b3, f)

    w1a = np.ascontiguousarray(W1.reshape(L, KC, 128, H1).transpose(2, 0, 1, 3))
    rwa = np.zeros((128, KC, 128), f)
    rwa[:, :, 0:NI] = router_W.T.reshape(KC, 128, NI).transpose(1, 0, 2)

    # W2 padded variants: e=0 -> [W2_a | 0], e=1 -> [0 | W2_b], so both
    # leaves of a pair accumulate into one [128,N] PSUM bank.
    w2a = np.zeros((NPAIR, 2, 128, 128), f)
    for s in range(NPAIR):
        w2a[s, 0, :, 0:64] = W2[2 * s]
        w2a[s, 1, :, 64:128] = W2[2 * s + 1]
    w2a = np.ascontiguousarray(w2a.transpose(2, 0, 1, 3))

    w3p = np.zeros((NPAIR, 128, 128), f)
    for s in range(NPAIR):
        w3p[s, 0:64, 0:8] = W3[2 * s]
        w3p[s, 64:128, 8:16] = W3[2 * s + 1]
    w3p = np.ascontiguousarray(w3p.transpose(1, 0, 2))

    a64 = np.zeros((128, L), f)
    for leaf in range(L):
        for row in _leaf_path_rows(leaf):
            a64[row, leaf] += 1.0

    # BSEL: broadcast pT [64 leaves] into the per-group pred slot layout
    # (32-row blocks per pair: 8 slots leaf_a, 8 slots leaf_b, 16 zeros).
    bsel = np.zeros((128, NG, 128), f)
    for g in range(NG):
        for jj in range(4):
            for m in range(16):
                leaf = 2 * (4 * g + jj) + (m >= 8)
                bsel[leaf, g, 32 * jj + m] = 1.0

    # R fold / b3 terms, padded to [128,128] with the real columns at
    # offset 32*t so both batch tiles share one PSUM accumulator bank.
    rsel2 = np.zeros((128, T, 128), f)
    for t in range(T):
        for kk in range(128):
            m = kk % 32
            if m < 8:
                rsel2[kk, t, 32 * t + m] = 1.0
            elif m < 16:
                rsel2[kk, t, 32 * t + (m - 8)] = 1.0
    b3e = np.zeros((128, T, 128), f)
    for t in range(T):
        b3e[0:L, t, 32 * t:32 * t + OUT] = b3

    return {
        "w1a": w1a.astype(b16),
        "rwa": rwa.astype(b16),
        "w2a": w2a.astype(b16),
        "w3p": w3p.astype(b16),
        "a64": a64,
        "bsel": bsel.astype(b16),
        "rsel2": rsel2.astype(b16),
        "b3e": b3e.astype(b16),
        "b1a": np.ascontiguousarray(b1.T),
        "b2a": np.ascontiguousarray(b2.reshape(NPAIR, 128).T),
        "rbp": np.concatenate([router_b, [0.0]]).astype(f)[:, None],
        "rbn": np.concatenate([-router_b, [0.0]]).astype(f)[:, None],
    }


def pack_x_core(x_core):
    """[1024, 512] slice -> [128, T, KC, 512] tile-major transposed bf16."""
    xc = np.asarray(x_core, np.float32)
    parts = []
    for t in range(T):
        parts.append(xc[N * t:N * (t + 1)].T.reshape(KC, 128, N))
    stacked = np.stack(parts, axis=0)            # [T, KC, 128, N]
    return np.ascontiguousarray(
        stacked.transpose(2, 0, 1, 3)).astype(ml_dtypes.bfloat16)


_NC_CACHE = {}


def _get_nc():
    if "nc" not in _NC_CACHE:
        _NC_CACHE["nc"] = build_nc()
    return _NC_CACHE["nc"]


def kernel(**inputs):
    x = np.asarray(inputs["x"], np.float32)
    shared = pack_shared(inputs["router_W"], inputs["router_b"],
                         inputs["W1"], inputs["b1"], inputs["W2"],
                         inputs["b2"], inputs["W3"], inputs["b3"])
    in_maps = []
    for i in range(NCORES):
        m = dict(shared)
        m["xa"] = pack_x_core(x[BC * i:BC * (i + 1)])
        in_maps.append(m)
    nc = _get_nc()
    res = run_bass_kernel_spmd(nc, in_maps, core_ids=list(range(NCORES)))
    out = np.concatenate([r["outT"].T for r in res.results], axis=0)
    return np.ascontiguousarray(out, np.float32)


# revision 15
# speedup vs baseline: 1.0305x; 1.0068x over previous
"""Trainium2 Bass kernel for NeuralDecisionTree (soft decision tree MoE).

Strategy: data-parallel over batch across 8 NeuronCores (1024 rows/core),
weights replicated.  All heavy GEMMs run in bf16 (~halves weight-load
time, DMA bytes and PE power vs fp32r; ~5e-3 rel error, well within
tolerance).

TRN2 PE pays a ~90-110ns reconfiguration stall whenever consecutive
matmuls change tile shape or dtype (measured: 128x128->128x128 issues at
~220ns, any shape/dtype change ~310-340ns).  So every stationary operand
in the steady state is zero-padded to a [128,128] bf16 tile:
  - W2 per-leaf variants [W2_a|0] / [0|W2_b] accumulate both leaves of a
    pair into one [128,N] PSUM bank (start=True / start=False).
  - W3 pair pack padded 32->128 cols (pred rows 16..127 are zeros).
  - router weights padded 63->128 cols.
  - leaf probabilities: pT = exp(A64 @ S) once per batch tile (the only
    2 fp32r matmuls), stored bf16 with rows 64..127 zeroed; per-group
    broadcast into the pred slot layout via 0/1 selection matmuls
    BSEL_g @ pT (bf16, exact).
  - output mix: R^T and b3^T padded to [128,128] with the real columns
    at offset 32*t, all accumulating into one PSUM bank held open across
    the whole kernel (b3 terms start the group, 16 R-folds accumulate).

Per-core dataflow (activations kept in [feature, batch] layout):
  router:  z = router_W @ x^T            (4 K-chunk bf16 matmuls)
  S      = [ln s; 0; ln(1-s); 0]         (128 rows; s = sigmoid(z + rb))
  pT     = exp(A @ S)                    [64 leaves, N] -> bf16
  L1:     h1T_l = relu(W1_l^T @ x^T + b1)  4 K-chunk bf16 matmuls -> ACT
  L2:     pair into one [128,N] PSUM bank, single DVE bias+relu -> bf16
  L3:     pred pair via padded block-diagonal W3 pack -> [128,N] PSUM
  mix:    prod = pred * broadcast(pT) (DVE), out_ps += R_t^T @ prod.
"""

import os
import sys

import numpy as np

if "/opt/trn_rl_repo" not in sys.path:
    sys.path.insert(0, "/opt/trn_rl_repo")

import ml_dtypes

import concourse.bass as bass
import concourse.hw_specs as hw_specs
import concourse.tile as tile
from concourse import bacc, mybir
from concourse.bass_utils import run_bass_kernel_spmd

_ONE_TABLE = "natural_log_exp_and_others"
_orig_get_tables = hw_specs.get_activation_tables


def _patched_get_tables(module_arch):
    """Confine activation-table choice to one set that covers every ACT
    func this kernel uses (exp/ln/relu/abs/copy/identity), so the greedy
    per-instruction table picker never ping-pongs between sets.  Dict
    order (= act_func_set_id) is preserved; other sets are emptied."""
    tables = dict(_orig_get_tables(module_arch))
    keep = tables[_ONE_TABLE]
    return {k: (v if k == _ONE_TABLE else (v & set()) or set())
            if k != _ONE_TABLE else keep for k, v in tables.items()}

f32 = mybir.dt.float32
f32r = mybir.dt.float32r
bf16 = mybir.dt.bfloat16
AF = mybir.ActivationFunctionType
ALU = mybir.AluOpType

# Problem shape (hardcoded; harness contract)
B = 8192
D = 512
H1 = 128
H2 = 64
OUT = 8
L = 64
NI = 63
NCORES = 8
BC = B // NCORES        # 1024 rows per core
N = 512                 # batch tile (matmul free dim / PSUM bank)
T = BC // N             # 2 batch tiles per core
KC = D // 128           # 4 contraction chunks for the input dim
NPAIR = L // 2          # 32 leaf pairs
NG = 8                  # 8-leaf groups


def _leaf_path_rows(leaf):
    """Rows of the [128] log-sigmoid stack contributing to log p(leaf).

    Row n (n<63) holds ln d_n; row 64+n holds ln(1-d_n); rows 63 and 127
    are zero pads.  Mirrors the reference's level-wise p interleave.
    """
    rows = []
    for k in range(6):
        prefix = leaf >> (6 - k)
        node = (2 ** k - 1) + prefix
        bit = (leaf >> (5 - k)) & 1
        rows.append(node + 64 * bit)
    return rows


def build_nc():
    nc = bacc.Bacc("TRN2", target_bir_lowering=False, debug=False,
                   num_devices=NCORES)
    bacc_mod = sys.modules["concourse.bacc"]
    bacc_mod.get_activation_tables = _patched_get_tables

    d_xa = nc.dram_tensor("xa", [128, T, KC, N], bf16, kind="ExternalInput").ap()
    d_w1 = nc.dram_tensor("w1a", [128, L, KC, 128], bf16, kind="ExternalInput").ap()
    d_rw = nc.dram_tensor("rwa", [128, KC, 128], bf16, kind="ExternalInput").ap()
    d_w2 = nc.dram_tensor("w2a", [128, NPAIR, 2, 128], bf16, kind="ExternalInput").ap()
    d_w3 = nc.dram_tensor("w3p", [128, NPAIR, 128], bf16, kind="ExternalInput").ap()
    d_a64 = nc.dram_tensor("a64", [128, L], f32r, kind="ExternalInput").ap()
    d_bsel = nc.dram_tensor("bsel", [128, NG, 128], bf16, kind="ExternalInput").ap()
    d_r = nc.dram_tensor("rsel2", [128, T, 128], bf16, kind="ExternalInput").ap()
    d_rn = nc.dram_tensor("rseln", [128, 32], bf16, kind="ExternalInput").ap()
    d_b3 = nc.dram_tensor("b3e", [128, T, 128], bf16, kind="ExternalInput").ap()
    d_b1 = nc.dram_tensor("b1a", [128, L], f32, kind="ExternalInput").ap()
    d_b2 = nc.dram_tensor("b2a", [128, NPAIR], f32, kind="ExternalInput").ap()
    d_rbp = nc.dram_tensor("rbp", [64, 1], f32, kind="ExternalInput").ap()
    d_rbn = nc.dram_tensor("rbn", [64, 1], f32, kind="ExternalInput").ap()
    d_out = nc.dram_tensor("outT", [OUT, BC], f32, kind="ExternalOutput").ap()

    with tile.TileContext(nc) as tc:
        with tc.tile_pool(name="const", bufs=1) as cpool, \
             tc.tile_pool(name="w1", bufs=2) as w1pool, \
             tc.tile_pool(name="w2w3", bufs=3) as h2wpool, \
             tc.tile_pool(name="spool", bufs=2) as spool, \
             tc.tile_pool(name="parr", bufs=3) as papool, \
             tc.tile_pool(name="h1", bufs=4) as h1pool, \
             tc.tile_pool(name="h2", bufs=3) as h2pool, \
             tc.tile_pool(name="prod", bufs=3) as prpool, \
             tc.tile_pool(name="osb", bufs=2) as opool, \
             tc.tile_pool(name="ps_h1", bufs=3, space="PSUM") as ps_h1, \
             tc.tile_pool(name="ps_h2", bufs=2, space="PSUM") as ps_h2, \
             tc.tile_pool(name="ps_pm", bufs=2, space="PSUM") as ps_pm, \
             tc.tile_pool(name="ps_out", bufs=1, space="PSUM") as ps_out:

            # ---- constants into SBUF, in byte-arrival order ----
            # The serial sync (hardware-DGE) queue carries the bulk in the
            # order the PE consumes it; the scalar hardware queue runs in
            # parallel with the tiny constants plus x's second batch tile
            # (so z/L1 on tile 1 aren't gated behind W1 bytes on sync).
            rwa = cpool.tile([128, KC, 128], bf16)
            nc.sync.dma_start(rwa[:], d_rw)
            xa = cpool.tile([128, T, KC, N], bf16)
            for c in range(KC):
                nc.sync.dma_start(xa[:, 0, c, :], d_xa[:, 0, c, :])
            w1g0 = w1pool.tile([128, 8, KC, 128], bf16, tag="w1", name="w1g0")
            nc.sync.dma_start(w1g0[:, 0:2, :, :], d_w1[:, 0:2, :, :])
            w2g0 = h2wpool.tile([128, 4, 2, 128], bf16, tag="w2", name="w2g0")
            nc.sync.dma_start(w2g0[:], d_w2[:, 0:4, :, :])
            w3g0 = h2wpool.tile([128, 4, 128], bf16, tag="w3", name="w3g0")
            nc.sync.dma_start(w3g0[:], d_w3[:, 0:4, :])
            a64 = cpool.tile([128, L], f32r)
            nc.sync.dma_start(a64[:], d_a64)
            bsel = cpool.tile([128, NG, 128], bf16)
            nc.sync.dma_start(bsel[:], d_bsel)
            nc.sync.dma_start(w1g0[:, 2:4, :, :], d_w1[:, 2:4, :, :])
            rsel2 = cpool.tile([128, T, 128], bf16)
            nc.sync.dma_start(rsel2[:], d_r)
            rseln = cpool.tile([128, 32], bf16)
            nc.sync.dma_start(rseln[:], d_rn)
            b3e = cpool.tile([128, T, 128], bf16)
            nc.sync.dma_start(b3e[:], d_b3)
            nc.sync.dma_start(w1g0[:, 4:6, :, :], d_w1[:, 4:6, :, :])
            nc.sync.dma_start(w1g0[:, 6:8, :, :], d_w1[:, 6:8, :, :])

            rbp = cpool.tile([64, 1], f32)
            nc.scalar.dma_start(rbp[:], d_rbp)
            rbn = cpool.tile([64, 1], f32)
            nc.scalar.dma_start(rbn[:], d_rbn)
            b1a = cpool.tile([128, L], f32)
            nc.scalar.dma_start(b1a[:], d_b1)
            for c in range(KC):
                nc.scalar.dma_start(xa[:, 1, c, :], d_xa[:, 1, c, :])
            b2a = cpool.tile([128, NPAIR], f32)
            nc.scalar.dma_start(b2a[:], d_b2)

            # ---- routing + first L1 pair, interleaved for startup ----
            # PE order: z(t0) -> L1 pair0 t0 (needs only the first W1 chunk)
            # -> z(t1) (by which time the second half of x has landed).
            # Row 63 of z is a zero pad from the padded router weights.
            #   ln s     = -(relu(-z') + ln(1 + exp(-|z'|)))
            #   ln (1-s) = -(relu( z') + ln(1 + exp(-|z'|)))
            def emit_l1_leaf(s, t, jj, e, w1g):
                leaf = 2 * s + e
                h1_ps = ps_h1.tile([128, N], f32, tag="h1",
                                   name=f"h1ps{s}_{t}_{e}")
                for c in range(KC):
                    nc.tensor.matmul(
                        h1_ps[:], w1g[:, 2 * jj + e, c, :],
                        xa[:, t, c, :],
                        start=(c == 0), stop=(c == KC - 1))
                h1_t = h1pool.tile([128, N], bf16, tag="h1s",
                                   name=f"h1s{s}_{t}_{e}")
                nc.scalar.activation(h1_t[:], h1_ps[:], AF.Relu,
                                     bias=b1a[:, leaf:leaf + 1],
                                     scale=1.0)
                return h1_t

            def emit_l1(s, t, jj, w1g):
                return [emit_l1_leaf(s, t, jj, e, w1g) for e in range(2)]

            z_pss = []
            pre_h1 = {}
            for t in range(T):
                z_ps = ps_pm.tile([128, N], f32, tag="pm", name=f"z_ps{t}")
                for c in range(KC):
                    nc.tensor.matmul(z_ps[:], rwa[:, c, :],
                                     xa[:, t, c, :],
                                     start=(c == 0), stop=(c == KC - 1))
                z_pss.append(z_ps)
                if t == 0:
                    pre_h1[(0, 0)] = emit_l1(0, 0, 0, w1g0)
            qs, rzps, rzns = [], [], []
            for t in range(T):
                z_ps = z_pss[t]
                az = spool.tile([64, N], f32, tag="az", name=f"az{t}")
                nc.scalar.activation(az[:], z_ps[0:64, :], AF.Abs,
                                     bias=rbp[:], scale=1.0)
                e_t = spool.tile([64, N], f32, tag="e", name=f"e{t}")
                nc.scalar.activation(e_t[:], az[:], AF.Exp, scale=-1.0)
                rzp = spool.tile([64, N], f32, tag="rzp", name=f"rzp{t}")
                nc.scalar.activation(rzp[:], z_ps[0:64, :], AF.Relu,
                                     bias=rbp[:], scale=1.0)
                rzn = spool.tile([64, N], f32, tag="rzn", name=f"rzn{t}")
                nc.scalar.activation(rzn[:], z_ps[0:64, :], AF.Relu,
                                     bias=rbn[:], scale=-1.0)
                qs.append(e_t)
                rzps.append(rzp)
                rzns.append(rzn)
            s_tiles = []
            for t in range(T):
                q_t = qs[t]
                nc.scalar.activation(q_t[:], q_t[:], AF.Ln, bias=1.0,
                                     scale=1.0)
                s_t = spool.tile([128, N], f32r, tag="s", name=f"s{t}")
                nc.vector.scalar_tensor_tensor(
                    s_t[0:64, :], rzns[t][:], -1.0, q_t[:],
                    op0=ALU.mult, op1=ALU.subtract)
                nc.vector.scalar_tensor_tensor(
                    s_t[64:128, :], rzps[t][:], -1.0, q_t[:],
                    op0=ALU.mult, op1=ALU.subtract)
                s_tiles.append(s_t)

            # ---- leaf probabilities: pT = exp(A @ S) once per batch tile
            # (the only fp32r matmuls in the kernel), held in bf16 with
            # rows 64..127 zeroed so the bf16 BSEL broadcasts can contract
            # the full 128 partitions.
            pt_tiles = []
            for t in range(T):
                pt_ext = spool.tile([128, N], bf16, tag="ptx", name=f"ptx{t}")
                nc.vector.memset(pt_ext[64:128, :], 0.0)
                pt_ps = ps_pm.tile([64, N], f32, tag="pm", name=f"pt_ps{t}")
                nc.tensor.matmul(pt_ps[:], a64[:], s_tiles[t][:],
                                 start=True, stop=True)
                nc.scalar.activation(pt_ext[0:64, :], pt_ps[:], AF.Exp,
                                     scale=1.0)
                pt_tiles.append(pt_ext)

            # ---- per-batch-tile output accumulator: one PSUM bank, t0's
            # real rows at partition base 0 and t1's at base 32 (the padded
            # R/b3 stationaries place their real columns at 32*t).  The
            # accumulation group stays open across all 8 groups; writes are
            # full-bank with zeros outside each tile's real rows, so
            # interleaving is harmless.  b3 terms run first: t0 start=True
            # zeroes the bank, t1 accumulates.
            out_ps = ps_out.tile([128, N], f32, tag="out", name="out_ps")

            # ---- main loop over 8-leaf groups ----
            def emit_p_block(g, t):
                """BSEL broadcast of pT into this group's pred slot layout
                (bf16 0/1 matmul, exact), copied to SBUF."""
                pa_ps = ps_pm.tile([128, N], f32, tag="pm",
                                   name=f"pa_ps{g}_{t}")
                nc.tensor.matmul(pa_ps[:], bsel[:, g, :], pt_tiles[t][:],
                                 start=True, stop=True)
                pa_t = papool.tile([128, N], bf16, tag="pa", name=f"pa{g}_{t}")
                nc.scalar.activation(pa_t[:], pa_ps[:], AF.Copy, scale=1.0)
                return pa_t

            first_rsel = [True, True]
            for g in range(NG):
                if g == 0:
                    w1g, w2g, w3g = w1g0, w2g0, w3g0
                else:
                    w2g = h2wpool.tile([128, 4, 2, 128], bf16, tag="w2",
                                       name=f"w2g{g}")
                    nc.sync.dma_start(w2g[:], d_w2[:, 4 * g:4 * (g + 1), :, :])
                    w3g = h2wpool.tile([128, 4, 128], bf16, tag="w3",
                                       name=f"w3g{g}")
                    nc.sync.dma_start(w3g[:], d_w3[:, 4 * g:4 * (g + 1), :])
                    w1g = w1pool.tile([128, 8, KC, 128], bf16, tag="w1",
                                      name=f"w1g{g}")
                    for hc in range(2):
                        nc.sync.dma_start(
                            w1g[:, 4 * hc:4 * (hc + 1), :, :],
                            d_w1[:, 8 * g + 4 * hc:8 * g + 4 * (hc + 1), :, :])
                # For g>0, p broadcasts can be computed up front (pT is
                # long done).  For g=0 they are emitted mid-way through the
                # first pair so the PE queue never waits on the routing
                # ACT/DVE chain.
                if g > 0:
                    pa_ts = [emit_p_block(g, t) for t in range(T)]
                else:
                    pa_ts = [None, None]
                prod_ts = [prpool.tile([128, N], bf16, tag="prod",
                                       name=f"prod{g}_{t}") for t in range(T)]
                def emit_l2(s, t, jj, h1_pair):
                    """Both leaves of the pair into one [128,N] PSUM bank
                    via padded [W2_a|0] / [0|W2_b] stationaries (PSUM
                    accumulation), one fused bias+relu DVE."""
                    h2_ps = ps_h2.tile([128, N], f32, tag="h2",
                                       name=f"h2ps{s}_{t}")
                    for e in range(2):
                        nc.tensor.matmul(h2_ps[:], w2g[:, jj, e, :],
                                         h1_pair[e][:],
                                         start=(e == 0), stop=(e == 1))
                    h2pair = h2pool.tile([128, N], bf16, tag="h2s",
                                         name=f"h2p{s}_{t}")
                    nc.vector.tensor_scalar(
                        h2pair[:], h2_ps[:], b2a[:, s:s + 1], 0.0,
                        op0=ALU.add, op1=ALU.max)
                    return h2pair

                def emit_l3(s, t, jj, h2pair):
                    pred_ps = ps_pm.tile([128, N], f32, tag="pm",
                                         name=f"pred{s}_{t}")
                    nc.tensor.matmul(pred_ps[:], w3g[:, jj, :], h2pair[:],
                                     start=True, stop=True)
                    if g == 0 and jj == 0:
                        pa_ts[t] = emit_p_block(0, t)
                        if t == 1:
                            # b3 terms open the out accumulation group as
                            # soon as both pT tiles exist (before any R
                            # fold): t0 zeroes the bank, t1 accumulates.
                            for tt in range(T):
                                nc.tensor.matmul(
                                    out_ps[:], b3e[:, tt, :],
                                    pt_tiles[tt][:],
                                    start=(tt == 0), stop=False,
                                    skip_group_check=True)
                    nc.vector.tensor_mul(
                        prod_ts[t][32 * jj:32 * (jj + 1), :],
                        pred_ps[0:32, :],
                        pa_ts[t][32 * jj:32 * (jj + 1), :])

                for jj in range(4):
                    s = 4 * g + jj
                    # pipelined order: L1(t0) -> L1(t1,A) -> L2(t0) ->
                    # L1(t1,B) -> L3(t0) -> L2(t1) -> L3(t1); keeps ACT's
                    # relus and DVE's h2 ops ahead of their PE consumers
                    # while holding at most 3 h1 PSUM banks.
                    if (s, 0) in pre_h1:
                        h1_t0 = pre_h1[(s, 0)]
                    else:
                        h1_t0 = emit_l1(s, 0, jj, w1g)
                    h1_t1 = [emit_l1_leaf(s, 1, jj, 0, w1g)]
                    h2_t0 = emit_l2(s, 0, jj, h1_t0)
                    h1_t1.append(emit_l1_leaf(s, 1, jj, 1, w1g))
                    emit_l3(s, 0, jj, h2_t0)
                    h2_t1 = emit_l2(s, 1, jj, h1_t1)
                    emit_l3(s, 1, jj, h2_t1)
                for t in range(T):
                    if g < NG - 1:
                        nc.tensor.matmul(out_ps[:], rsel2[:, t, :],
                                         prod_ts[t][:],
                                         start=False, stop=False,
                                         skip_group_check=True)
                    else:
                        # Final fold per batch tile is narrow (writes only
                        # its own 32 partition rows), so t0's output copy +
                        # DMA overlap t1's remaining mix chain.
                        nc.tensor.matmul(out_ps[32 * t:32 * t + 32, :],
                                         rseln[:], prod_ts[t][:],
                                         start=False, stop=True,
                                         skip_group_check=True)
                        out_sb = opool.tile([OUT, N], f32, tag="o",
                                            name=f"osb{t}")
                        nc.scalar.activation(
                            out_sb[:], out_ps[32 * t:32 * t + OUT, :],
                            AF.Copy)
                        nc.sync.dma_start(d_out[:, N * t:N * (t + 1)],
                                          out_sb[:])

    try:
        nc.compile()
    finally:
        bacc_mod.get_activation_tables = _orig_get_tables
    return nc


def pack_shared(router_W, router_b, W1, b1, W2, b2, W3, b3):
    """Host-side packing of replicated parameters into SBUF-friendly layouts."""
    f = np.float32
    b16 = ml_dtypes.bfloat16
    router_W = np.asarray(router_W, f)
    router_b = np.asarray(router_b, f)
    W1 = np.asarray(W1, f)
    b1 = np.asarray(b1, f)
    W2 = np.asarray(W2, f)
    b2 = np.asarray(b2, f)
    W3 = np.asarray(W3, f)
    b3 = np.asarray(b3, f)

    w1a = np.ascontiguousarray(W1.reshape(L, KC, 128, H1).transpose(2, 0, 1, 3))
    rwa = np.zeros((128, KC, 128), f)
    rwa[:, :, 0:NI] = router_W.T.reshape(KC, 128, NI).transpose(1, 0, 2)

    # W2 padded variants: e=0 -> [W2_a | 0], e=1 -> [0 | W2_b], so both
    # leaves of a pair accumulate into one [128,N] PSUM bank.
    w2a = np.zeros((NPAIR, 2, 128, 128), f)
    for s in range(NPAIR):
        w2a[s, 0, :, 0:64] = W2[2 * s]
        w2a[s, 1, :, 64:128] = W2[2 * s + 1]
    w2a = np.ascontiguousarray(w2a.transpose(2, 0, 1, 3))

    w3p = np.zeros((NPAIR, 128, 128), f)
    for s in range(NPAIR):
        w3p[s, 0:64, 0:8] = W3[2 * s]
        w3p[s, 64:128, 8:16] = W3[2 * s + 1]
    w3p = np.ascontiguousarray(w3p.transpose(1, 0, 2))

    a64 = np.zeros((128, L), f)
    for leaf in range(L):
        for row in _leaf_path_rows(leaf):
            a64[row, leaf] += 1.0

    # BSEL: broadcast pT [64 leaves] into the per-group pred slot layout
    # (32-row blocks per pair: 8 slots leaf_a, 8 slots leaf_b, 16 zeros).
    bsel = np.zeros((128, NG, 128), f)
    for g in range(NG):
        for jj in range(4):
            for m in range(16):
                leaf = 2 * (4 * g + jj) + (m >= 8)
                bsel[leaf, g, 32 * jj + m] = 1.0

    # R fold / b3 terms, padded to [128,128] with the real columns at
    # offset 32*t so both batch tiles share one PSUM accumulator bank.
    rsel2 = np.zeros((128, T, 128), f)
    for t in range(T):
        for kk in range(128):
            m = kk % 32
            if m < 8:
                rsel2[kk, t, 32 * t + m] = 1.0
            elif m < 16:
                rsel2[kk, t, 32 * t + (m - 8)] = 1.0
    rseln = np.zeros((128, 32), f)
    for kk in range(128):
        m = kk % 32
        if m < 8:
            rseln[kk, m] = 1.0
        elif m < 16:
            rseln[kk, m - 8] = 1.0
    b3e = np.zeros((128, T, 128), f)
    for t in range(T):
        b3e[0:L, t, 32 * t:32 * t + OUT] = b3

    return {
        "w1a": w1a.astype(b16),
        "rwa": rwa.astype(b16),
        "w2a": w2a.astype(b16),
        "w3p": w3p.astype(b16),
        "a64": a64,
        "bsel": bsel.astype(b16),
        "rsel2": rsel2.astype(b16),
        "rseln": rseln.astype(b16),
        "b3e": b3e.astype(b16),
        "b1a": np.ascontiguousarray(b1.T),
        "b2a": np.ascontiguousarray(b2.reshape(NPAIR, 128).T),
        "rbp": np.concatenate([router_b, [0.0]]).astype(f)[:, None],
        "rbn": np.concatenate([-router_b, [0.0]]).astype(f)[:, None],
    }


def pack_x_core(x_core):
    """[1024, 512] slice -> [128, T, KC, 512] tile-major transposed bf16."""
    xc = np.asarray(x_core, np.float32)
    parts = []
    for t in range(T):
        parts.append(xc[N * t:N * (t + 1)].T.reshape(KC, 128, N))
    stacked = np.stack(parts, axis=0)            # [T, KC, 128, N]
    return np.ascontiguousarray(
        stacked.transpose(2, 0, 1, 3)).astype(ml_dtypes.bfloat16)


_NC_CACHE = {}


def _get_nc():
    if "nc" not in _NC_CACHE:
        _NC_CACHE["nc"] = build_nc()
    return _NC_CACHE["nc"]


def kernel(**inputs):
    x = np.asarray(inputs["x"], np.float32)
    shared = pack_shared(inputs["router_W"], inputs["router_b"],
                         inputs["W1"], inputs["b1"], inputs["W2"],
                         inputs["b2"], inputs["W3"], inputs["b3"])
    in_maps = []
    for i in range(NCORES):
        m = dict(shared)
        m["xa"] = pack_x_core(x[BC * i:BC * (i + 1)])
        in_maps.append(m)
    nc = _get_nc()
    res = run_bass_kernel_spmd(nc, in_maps, core_ids=list(range(NCORES)))
    out = np.concatenate([r["outT"].T for r in res.results], axis=0)
    return np.ascontiguousarray(out, np.float32)


# revision 18
# speedup vs baseline: 1.0443x; 1.0134x over previous
"""Trainium2 Bass kernel for NeuralDecisionTree (soft decision tree MoE).

Strategy: data-parallel over batch across 8 NeuronCores (1024 rows/core),
weights replicated.  All heavy GEMMs run in bf16 (~halves weight-load
time, DMA bytes and PE power vs fp32r; ~5e-3 rel error, well within
tolerance).

TRN2 PE pays a ~90-110ns reconfiguration stall whenever consecutive
matmuls change tile shape or dtype (measured: 128x128->128x128 issues at
~220ns, any shape/dtype change ~310-340ns).  So every stationary operand
in the steady state is zero-padded to a [128,128] bf16 tile:
  - W2 per-leaf variants [W2_a|0] / [0|W2_b] accumulate both leaves of a
    pair into one [128,N] PSUM bank (start=True / start=False).
  - W3 pair pack padded 32->128 cols (pred rows 16..127 are zeros).
  - router weights padded 63->128 cols.
  - leaf probabilities: pT = exp(A64 @ S) once per batch tile (the only
    2 fp32r matmuls), stored bf16 with rows 64..127 zeroed; per-group
    broadcast into the pred slot layout via 0/1 selection matmuls
    BSEL_g @ pT (bf16, exact).
  - output mix: R^T and b3^T padded to [128,128] with the real columns
    at offset 32*t, all accumulating into one PSUM bank held open across
    the whole kernel (b3 terms start the group, 16 R-folds accumulate).

Per-core dataflow (activations kept in [feature, batch] layout):
  router:  z = router_W @ x^T            (4 K-chunk bf16 matmuls)
  S      = [ln s; 0; ln(1-s); 0]         (128 rows; s = sigmoid(z + rb))
  pT     = exp(A @ S)                    [64 leaves, N] -> bf16
  L1:     h1T_l = relu(W1_l^T @ x^T + b1)  4 K-chunk bf16 matmuls -> ACT
  L2:     pair into one [128,N] PSUM bank, single DVE bias+relu -> bf16
  L3:     pred pair via padded block-diagonal W3 pack -> [128,N] PSUM
  mix:    prod = pred * broadcast(pT) (DVE), out_ps += R_t^T @ prod.
"""

import os
import sys

import numpy as np

if "/opt/trn_rl_repo" not in sys.path:
    sys.path.insert(0, "/opt/trn_rl_repo")

import ml_dtypes

import concourse.bass as bass
import concourse.hw_specs as hw_specs
import concourse.tile as tile
from concourse import bacc, mybir
from concourse.bass_utils import run_bass_kernel_spmd

_ONE_TABLE = "natural_log_exp_and_others"
_orig_get_tables = hw_specs.get_activation_tables


def _patched_get_tables(module_arch):
    """Confine activation-table choice to one set that covers every ACT
    func this kernel uses (exp/ln/relu/abs/copy/identity), so the greedy
    per-instruction table picker never ping-pongs between sets.  Dict
    order (= act_func_set_id) is preserved; other sets are emptied."""
    tables = dict(_orig_get_tables(module_arch))
    keep = tables[_ONE_TABLE]
    return {k: (v if k == _ONE_TABLE else (v & set()) or set())
            if k != _ONE_TABLE else keep for k, v in tables.items()}

f32 = mybir.dt.float32
f32r = mybir.dt.float32r
bf16 = mybir.dt.bfloat16
AF = mybir.ActivationFunctionType
ALU = mybir.AluOpType

# Problem shape (hardcoded; harness contract)
B = 8192
D = 512
H1 = 128
H2 = 64
OUT = 8
L = 64
NI = 63
NCORES = 8
BC = B // NCORES        # 1024 rows per core
N = 512                 # batch tile (matmul free dim / PSUM bank)
T = BC // N             # 2 batch tiles per core
KC = D // 128           # 4 contraction chunks for the input dim
NPAIR = L // 2          # 32 leaf pairs
NG = 8                  # 8-leaf groups


def _leaf_path_rows(leaf):
    """Rows of the [128] log-sigmoid stack contributing to log p(leaf).

    Row n (n<63) holds ln d_n; row 64+n holds ln(1-d_n); rows 63 and 127
    are zero pads.  Mirrors the reference's level-wise p interleave.
    """
    rows = []
    for k in range(6):
        prefix = leaf >> (6 - k)
        node = (2 ** k - 1) + prefix
        bit = (leaf >> (5 - k)) & 1
        rows.append(node + 64 * bit)
    return rows


def build_nc():
    nc = bacc.Bacc("TRN2", target_bir_lowering=False, debug=False,
                   num_devices=NCORES)
    bacc_mod = sys.modules["concourse.bacc"]
    bacc_mod.get_activation_tables = _patched_get_tables

    d_xa = nc.dram_tensor("xa", [128, T, KC, N], bf16, kind="ExternalInput").ap()
    d_w1 = nc.dram_tensor("w1a", [128, L, KC, 128], bf16, kind="ExternalInput").ap()
    d_rw = nc.dram_tensor("rwa", [128, KC, 128], bf16, kind="ExternalInput").ap()
    d_w2 = nc.dram_tensor("w2a", [128, NPAIR, 2, 128], bf16, kind="ExternalInput").ap()
    d_w3 = nc.dram_tensor("w3p", [128, NPAIR, 128], bf16, kind="ExternalInput").ap()
    d_a64 = nc.dram_tensor("a64", [128, L], f32r, kind="ExternalInput").ap()
    d_bsel = nc.dram_tensor("bsel", [128, NG, 128], bf16, kind="ExternalInput").ap()
    d_r = nc.dram_tensor("rsel2", [128, T, 128], bf16, kind="ExternalInput").ap()
    d_rn = nc.dram_tensor("rseln", [128, 32], bf16, kind="ExternalInput").ap()
    d_b3 = nc.dram_tensor("b3e", [128, T, 128], bf16, kind="ExternalInput").ap()
    d_b1 = nc.dram_tensor("b1a", [128, L], f32, kind="ExternalInput").ap()
    d_b2 = nc.dram_tensor("b2a", [128, NPAIR], f32, kind="ExternalInput").ap()
    d_rbp = nc.dram_tensor("rbp", [64, 1], f32, kind="ExternalInput").ap()
    d_rbn = nc.dram_tensor("rbn", [64, 1], f32, kind="ExternalInput").ap()
    d_out = nc.dram_tensor("outT", [OUT, BC], f32, kind="ExternalOutput").ap()

    with tile.TileContext(nc) as tc:
        with tc.tile_pool(name="const", bufs=1) as cpool, \
             tc.tile_pool(name="w1", bufs=2) as w1pool, \
             tc.tile_pool(name="w2w3", bufs=3) as h2wpool, \
             tc.tile_pool(name="spool", bufs=2) as spool, \
             tc.tile_pool(name="parr", bufs=4) as papool, \
             tc.tile_pool(name="h1", bufs=6) as h1pool, \
             tc.tile_pool(name="h2", bufs=3) as h2pool, \
             tc.tile_pool(name="prod", bufs=4) as prpool, \
             tc.tile_pool(name="osb", bufs=2) as opool, \
             tc.tile_pool(name="ps_h1", bufs=3, space="PSUM") as ps_h1, \
             tc.tile_pool(name="ps_h2", bufs=2, space="PSUM") as ps_h2, \
             tc.tile_pool(name="ps_pm", bufs=2, space="PSUM") as ps_pm, \
             tc.tile_pool(name="ps_out", bufs=1, space="PSUM") as ps_out:

            # ---- constants into SBUF, in byte-arrival order ----
            # The serial sync (hardware-DGE) queue carries the bulk in the
            # order the PE consumes it; the scalar hardware queue runs in
            # parallel with the tiny constants plus x's second batch tile
            # (so z/L1 on tile 1 aren't gated behind W1 bytes on sync).
            rwa = cpool.tile([128, KC, 128], bf16)
            nc.sync.dma_start(rwa[:], d_rw)
            xa = cpool.tile([128, T, KC, N], bf16)
            for c in range(KC):
                nc.sync.dma_start(xa[:, 0, c, :], d_xa[:, 0, c, :])
            w1g0 = w1pool.tile([128, 8, KC, 128], bf16, tag="w1", name="w1g0")
            nc.sync.dma_start(w1g0[:, 0:2, :, :], d_w1[:, 0:2, :, :])
            w2g0 = h2wpool.tile([128, 4, 2, 128], bf16, tag="w2", name="w2g0")
            nc.sync.dma_start(w2g0[:], d_w2[:, 0:4, :, :])
            w3g0 = h2wpool.tile([128, 4, 128], bf16, tag="w3", name="w3g0")
            nc.sync.dma_start(w3g0[:], d_w3[:, 0:4, :])
            a64 = cpool.tile([128, L], f32r)
            nc.sync.dma_start(a64[:], d_a64)
            bsel = cpool.tile([128, NG, 128], bf16)
            nc.sync.dma_start(bsel[:], d_bsel)
            nc.sync.dma_start(w1g0[:, 2:4, :, :], d_w1[:, 2:4, :, :])
            rsel2 = cpool.tile([128, T, 128], bf16)
            nc.sync.dma_start(rsel2[:], d_r)
            rseln = cpool.tile([128, 32], bf16)
            nc.sync.dma_start(rseln[:], d_rn)
            b3e = cpool.tile([128, T, 128], bf16)
            nc.sync.dma_start(b3e[:], d_b3)
            nc.sync.dma_start(w1g0[:, 4:6, :, :], d_w1[:, 4:6, :, :])
            nc.sync.dma_start(w1g0[:, 6:8, :, :], d_w1[:, 6:8, :, :])

            rbp = cpool.tile([64, 1], f32)
            nc.scalar.dma_start(rbp[:], d_rbp)
            rbn = cpool.tile([64, 1], f32)
            nc.scalar.dma_start(rbn[:], d_rbn)
            b1a = cpool.tile([128, L], f32)
            nc.scalar.dma_start(b1a[:], d_b1)
            for c in range(KC):
                nc.scalar.dma_start(xa[:, 1, c, :], d_xa[:, 1, c, :])
            b2a = cpool.tile([128, NPAIR], f32)
            nc.scalar.dma_start(b2a[:], d_b2)

            # ---- routing + first L1 pair, interleaved for startup ----
            # PE order: z(t0) -> L1 pair0 t0 (needs only the first W1 chunk)
            # -> z(t1) (by which time the second half of x has landed).
            # Row 63 of z is a zero pad from the padded router weights.
            #   ln s     = -(relu(-z') + ln(1 + exp(-|z'|)))
            #   ln (1-s) = -(relu( z') + ln(1 + exp(-|z'|)))
            def emit_l1_leaf(s, t, jj, e, w1g):
                leaf = 2 * s + e
                h1_ps = ps_h1.tile([128, N], f32, tag="h1",
                                   name=f"h1ps{s}_{t}_{e}")
                for c in range(KC):
                    nc.tensor.matmul(
                        h1_ps[:], w1g[:, 2 * jj + e, c, :],
                        xa[:, t, c, :],
                        start=(c == 0), stop=(c == KC - 1))
                h1_t = h1pool.tile([128, N], bf16, tag="h1s",
                                   name=f"h1s{s}_{t}_{e}")
                nc.scalar.activation(h1_t[:], h1_ps[:], AF.Relu,
                                     bias=b1a[:, leaf:leaf + 1],
                                     scale=1.0)
                return h1_t

            def emit_l1(s, t, jj, w1g):
                return [emit_l1_leaf(s, t, jj, e, w1g) for e in range(2)]

            z_pss = []
            pre_h1 = {}
            for t in range(T):
                z_ps = ps_pm.tile([128, N], f32, tag="pm", name=f"z_ps{t}")
                for c in range(KC):
                    nc.tensor.matmul(z_ps[:], rwa[:, c, :],
                                     xa[:, t, c, :],
                                     start=(c == 0), stop=(c == KC - 1))
                z_pss.append(z_ps)
                if t == 0:
                    pre_h1[(0, 0)] = emit_l1(0, 0, 0, w1g0)
            qs, rzps, rzns = [], [], []
            for t in range(T):
                z_ps = z_pss[t]
                az = spool.tile([64, N], f32, tag="az", name=f"az{t}")
                nc.scalar.activation(az[:], z_ps[0:64, :], AF.Abs,
                                     bias=rbp[:], scale=1.0)
                e_t = spool.tile([64, N], f32, tag="e", name=f"e{t}")
                nc.scalar.activation(e_t[:], az[:], AF.Exp, scale=-1.0)
                rzp = spool.tile([64, N], f32, tag="rzp", name=f"rzp{t}")
                nc.scalar.activation(rzp[:], z_ps[0:64, :], AF.Relu,
                                     bias=rbp[:], scale=1.0)
                rzn = spool.tile([64, N], f32, tag="rzn", name=f"rzn{t}")
                nc.scalar.activation(rzn[:], z_ps[0:64, :], AF.Relu,
                                     bias=rbn[:], scale=-1.0)
                qs.append(e_t)
                rzps.append(rzp)
                rzns.append(rzn)
            s_tiles = []
            for t in range(T):
                q_t = qs[t]
                nc.scalar.activation(q_t[:], q_t[:], AF.Ln, bias=1.0,
                                     scale=1.0)
                s_t = spool.tile([128, N], f32r, tag="s", name=f"s{t}")
                nc.vector.scalar_tensor_tensor(
                    s_t[0:64, :], rzns[t][:], -1.0, q_t[:],
                    op0=ALU.mult, op1=ALU.subtract)
                nc.vector.scalar_tensor_tensor(
                    s_t[64:128, :], rzps[t][:], -1.0, q_t[:],
                    op0=ALU.mult, op1=ALU.subtract)
                s_tiles.append(s_t)

            # ---- leaf probabilities: pT = exp(A @ S) once per batch tile
            # (the only fp32r matmuls in the kernel), held in bf16 with
            # rows 64..127 zeroed so the bf16 BSEL broadcasts can contract
            # the full 128 partitions.
            pt_tiles = []
            for t in range(T):
                pt_ext = spool.tile([128, N], bf16, tag="ptx", name=f"ptx{t}")
                nc.vector.memset(pt_ext[64:128, :], 0.0)
                pt_ps = ps_pm.tile([64, N], f32, tag="pm", name=f"pt_ps{t}")
                nc.tensor.matmul(pt_ps[:], a64[:], s_tiles[t][:],
                                 start=True, stop=True)
                nc.scalar.activation(pt_ext[0:64, :], pt_ps[:], AF.Exp,
                                     scale=1.0)
                pt_tiles.append(pt_ext)

            # ---- per-batch-tile output accumulator: one PSUM bank, t0's
            # real rows at partition base 0 and t1's at base 32 (the padded
            # R/b3 stationaries place their real columns at 32*t).  The
            # accumulation group stays open across all 8 groups; writes are
            # full-bank with zeros outside each tile's real rows, so
            # interleaving is harmless.  b3 terms run first: t0 start=True
            # zeroes the bank, t1 accumulates.
            out_ps = ps_out.tile([128, N], f32, tag="out", name="out_ps")

            # ---- main loop over 8-leaf groups ----
            def emit_p_block(g, t):
                """BSEL broadcast of pT into this group's pred slot layout
                (bf16 0/1 matmul, exact), copied to SBUF."""
                pa_ps = ps_pm.tile([128, N], f32, tag="pm",
                                   name=f"pa_ps{g}_{t}")
                nc.tensor.matmul(pa_ps[:], bsel[:, g, :], pt_tiles[t][:],
                                 start=True, stop=True)
                pa_t = papool.tile([128, N], bf16, tag="pa", name=f"pa{g}_{t}")
                nc.scalar.activation(pa_t[:], pa_ps[:], AF.Copy, scale=1.0)
                return pa_t

            # ---- main loop: flat lag-1 software pipeline over the 32
            # pairs.  Per pair s the PE emits
            #   L1(t0,s) | L2(t1,s-1) L3(t0,s-1) | L1(t1,s) | L2(t0,s)
            #   L3(t1,s-1)
            # so every L2/L3 has >=11 matmuls (~2.3us) of lead over the
            # ACT relu / DVE bias+relu that produces its moving operand
            # (the un-lagged order stalled ~126ns per L2(t1) waiting on
            # the h1 relu).  R folds trail similarly.
            wg = {}          # g -> (w1g, w2g, w3g)
            pa_all = {}      # g -> [pa_t0, pa_t1]
            prod_all = {}    # g -> [prod_t0, prod_t1]
            h2_done = {}     # (s, t) -> h2pair sbuf tile

            def emit_l2(s, t, h1_pair):
                """Both leaves of the pair into one [128,N] PSUM bank via
                padded [W2_a|0] / [0|W2_b] stationaries (PSUM
                accumulation), one fused bias+relu DVE."""
                jj = s % 4
                w2g = wg[s // 4][1]
                h2_ps = ps_h2.tile([128, N], f32, tag="h2",
                                   name=f"h2ps{s}_{t}")
                for e in range(2):
                    nc.tensor.matmul(h2_ps[:], w2g[:, jj, e, :],
                                     h1_pair[e][:],
                                     start=(e == 0), stop=(e == 1))
                h2pair = h2pool.tile([128, N], bf16, tag="h2s",
                                     name=f"h2p{s}_{t}")
                nc.vector.tensor_scalar(
                    h2pair[:], h2_ps[:], b2a[:, s:s + 1], 0.0,
                    op0=ALU.add, op1=ALU.max)
                h2_done[(s, t)] = h2pair

            def emit_l3(s, t):
                jj = s % 4
                w3g = wg[s // 4][2]
                pred_ps = ps_pm.tile([128, N], f32, tag="pm",
                                     name=f"pred{s}_{t}")
                nc.tensor.matmul(pred_ps[:], w3g[:, jj, :],
                                 h2_done.pop((s, t))[:],
                                 start=True, stop=True)
                nc.vector.tensor_mul(
                    prod_all[s // 4][t][32 * jj:32 * (jj + 1), :],
                    pred_ps[0:32, :],
                    pa_all[s // 4][t][32 * jj:32 * (jj + 1), :])

            def emit_rsel(g, t):
                nc.tensor.matmul(out_ps[:], rsel2[:, t, :],
                                 prod_all[g][t][:],
                                 start=False, stop=False,
                                 skip_group_check=True)

            h1_t0 = h1_t1 = None
            for s in range(NPAIR):
                g = s // 4
                if s % 4 == 0:
                    # group setup: weight DMAs + p broadcasts + prod tiles
                    if g == 0:
                        wg[0] = (w1g0, w2g0, w3g0)
                    else:
                        w2g = h2wpool.tile([128, 4, 2, 128], bf16, tag="w2",
                                           name=f"w2g{g}")
                        nc.sync.dma_start(w2g[:],
                                          d_w2[:, 4 * g:4 * (g + 1), :, :])
                        w3g = h2wpool.tile([128, 4, 128], bf16, tag="w3",
                                           name=f"w3g{g}")
                        nc.sync.dma_start(w3g[:], d_w3[:, 4 * g:4 * (g + 1), :])
                        w1g = w1pool.tile([128, 8, KC, 128], bf16, tag="w1",
                                          name=f"w1g{g}")
                        for hc in range(2):
                            nc.sync.dma_start(
                                w1g[:, 4 * hc:4 * (hc + 1), :, :],
                                d_w1[:, 8 * g + 4 * hc:
                                     8 * g + 4 * (hc + 1), :, :])
                        wg[g] = (w1g, w2g, w3g)
                        pa_all[g] = [emit_p_block(g, t) for t in range(T)]
                    prod_all[g] = [prpool.tile([128, N], bf16, tag="prod",
                                               name=f"prod{g}_{t}")
                                   for t in range(T)]
                w1g = wg[g][0]
                # -- L1(t0, s)
                if (s, 0) in pre_h1:
                    h1_next_t0 = pre_h1[(s, 0)]
                else:
                    h1_next_t0 = emit_l1(s, 0, s % 4, w1g)
                if s == 1:
                    # deferred g=0 p broadcasts + b3 accumulation openers:
                    # pT's exp chain is safely done by now, and the first R
                    # fold (s=4) comes much later.
                    pa_all[0] = [emit_p_block(0, t) for t in range(T)]
                    for tt in range(T):
                        nc.tensor.matmul(out_ps[:], b3e[:, tt, :],
                                         pt_tiles[tt][:],
                                         start=(tt == 0), stop=False,
                                         skip_group_check=True)
                if s > 0:
                    emit_l2(s - 1, 1, h1_t1)
                    emit_l3(s - 1, 0)
                    if s % 4 == 1 and s >= 5:
                        emit_rsel((s - 1) // 4 - 1, 1)
                h1_t0 = h1_next_t0
                # -- L1(t1, s)
                h1_t1 = emit_l1(s, 1, s % 4, w1g)
                emit_l2(s, 0, h1_t0)
                if s > 0:
                    emit_l3(s - 1, 1)
                if s % 4 == 0 and s >= 4:
                    emit_rsel(g - 1, 0)

            # ---- pipeline flush + output ----
            emit_l2(NPAIR - 1, 1, h1_t1)
            emit_l3(NPAIR - 1, 0)
            emit_l3(NPAIR - 1, 1)
            for t in range(T):
                # Final fold per batch tile is narrow (writes only its own
                # 32 partition rows), so t0's output copy + DMA overlap
                # t1's remaining mix chain.
                nc.tensor.matmul(out_ps[32 * t:32 * t + 32, :],
                                 rseln[:], prod_all[NG - 1][t][:],
                                 start=False, stop=True,
                                 skip_group_check=True)
                out_sb = opool.tile([OUT, N], f32, tag="o", name=f"osb{t}")
                nc.scalar.activation(out_sb[:],
                                     out_ps[32 * t:32 * t + OUT, :],
                                     AF.Copy)
                nc.sync.dma_start(d_out[:, N * t:N * (t + 1)], out_sb[:])

    try:
        nc.compile()
    finally:
        bacc_mod.get_activation_tables = _orig_get_tables
    return nc


def pack_shared(router_W, router_b, W1, b1, W2, b2, W3, b3):
    """Host-side packing of replicated parameters into SBUF-friendly layouts."""
    f = np.float32
    b16 = ml_dtypes.bfloat16
    router_W = np.asarray(router_W, f)
    router_b = np.asarray(router_b, f)
    W1 = np.asarray(W1, f)
    b1 = np.asarray(b1, f)
    W2 = np.asarray(W2, f)
    b2 = np.asarray(b2, f)
    W3 = np.asarray(W3, f)
    b3 = np.asarray(b3, f)

    w1a = np.ascontiguousarray(W1.reshape(L, KC, 128, H1).transpose(2, 0, 1, 3))
    rwa = np.zeros((128, KC, 128), f)
    rwa[:, :, 0:NI] = router_W.T.reshape(KC, 128, NI).transpose(1, 0, 2)

    # W2 padded variants: e=0 -> [W2_a | 0], e=1 -> [0 | W2_b], so both
    # leaves of a pair accumulate into one [128,N] PSUM bank.
    w2a = np.zeros((NPAIR, 2, 128, 128), f)
    for s in range(NPAIR):
        w2a[s, 0, :, 0:64] = W2[2 * s]
        w2a[s, 1, :, 64:128] = W2[2 * s + 1]
    w2a = np.ascontiguousarray(w2a.transpose(2, 0, 1, 3))

    w3p = np.zeros((NPAIR, 128, 128), f)
    for s in range(NPAIR):
        w3p[s, 0:64, 0:8] = W3[2 * s]
        w3p[s, 64:128, 8:16] = W3[2 * s + 1]
    w3p = np.ascontiguousarray(w3p.transpose(1, 0, 2))

    a64 = np.zeros((128, L), f)
    for leaf in range(L):
        for row in _leaf_path_rows(leaf):
            a64[row, leaf] += 1.0

    # BSEL: broadcast pT [64 leaves] into the per-group pred slot layout
    # (32-row blocks per pair: 8 slots leaf_a, 8 slots leaf_b, 16 zeros).
    bsel = np.zeros((128, NG, 128), f)
    for g in range(NG):
        for jj in range(4):
            for m in range(16):
                leaf = 2 * (4 * g + jj) + (m >= 8)
                bsel[leaf, g, 32 * jj + m] = 1.0

    # R fold / b3 terms, padded to [128,128] with the real columns at
    # offset 32*t so both batch tiles share one PSUM accumulator bank.
    rsel2 = np.zeros((128, T, 128), f)
    for t in range(T):
        for kk in range(128):
            m = kk % 32
            if m < 8:
                rsel2[kk, t, 32 * t + m] = 1.0
            elif m < 16:
                rsel2[kk, t, 32 * t + (m - 8)] = 1.0
    rseln = np.zeros((128, 32), f)
    for kk in range(128):
        m = kk % 32
        if m < 8:
            rseln[kk, m] = 1.0
        elif m < 16:
            rseln[kk, m - 8] = 1.0
    b3e = np.zeros((128, T, 128), f)
    for t in range(T):
        b3e[0:L, t, 32 * t:32 * t + OUT] = b3

    return {
        "w1a": w1a.astype(b16),
        "rwa": rwa.astype(b16),
        "w2a": w2a.astype(b16),
        "w3p": w3p.astype(b16),
        "a64": a64,
        "bsel": bsel.astype(b16),
        "rsel2": rsel2.astype(b16),
        "rseln": rseln.astype(b16),
        "b3e": b3e.astype(b16),
        "b1a": np.ascontiguousarray(b1.T),
        "b2a": np.ascontiguousarray(b2.reshape(NPAIR, 128).T),
        "rbp": np.concatenate([router_b, [0.0]]).astype(f)[:, None],
        "rbn": np.concatenate([-router_b, [0.0]]).astype(f)[:, None],
    }


def pack_x_core(x_core):
    """[1024, 512] slice -> [128, T, KC, 512] tile-major transposed bf16."""
    xc = np.asarray(x_core, np.float32)
    parts = []
    for t in range(T):
        parts.append(xc[N * t:N * (t + 1)].T.reshape(KC, 128, N))
    stacked = np.stack(parts, axis=0)            # [T, KC, 128, N]
    return np.ascontiguousarray(
        stacked.transpose(2, 0, 1, 3)).astype(ml_dtypes.bfloat16)


_NC_CACHE = {}


def _get_nc():
    if "nc" not in _NC_CACHE:
        _NC_CACHE["nc"] = build_nc()
    return _NC_CACHE["nc"]


def kernel(**inputs):
    x = np.asarray(inputs["x"], np.float32)
    shared = pack_shared(inputs["router_W"], inputs["router_b"],
                         inputs["W1"], inputs["b1"], inputs["W2"],
                         inputs["b2"], inputs["W3"], inputs["b3"])
    in_maps = []
    for i in range(NCORES):
        m = dict(shared)
        m["xa"] = pack_x_core(x[BC * i:BC * (i + 1)])
        in_maps.append(m)
    nc = _get_nc()
    res = run_bass_kernel_spmd(nc, in_maps, core_ids=list(range(NCORES)))
    out = np.concatenate([r["outT"].T for r in res.results], axis=0)
    return np.ascontiguousarray(out, np.float32)
